# revision 1
# baseline (speedup 1.0000x reference)
"""Trainium2 Bass kernel for nn_DecoderCell (LFADS decoder cell).

Strategy: pure data parallel over 8 NeuronCores (8192 batch rows each).
On-chip layout is fully transposed ([feature, batch]): batch rides the free
dim (512-wide tiles), gate features ride the partitions. All matmuls are
fp32r (full-rate at free-dim >= 256) with the small weights stationary
(host pre-transposed) and activations streaming. Biases are folded into the
matmuls via ones-row augmentation of the K dim. Sigmoid is synthesized from
tanh (one ACT table set: Exp+Tanh) with the affine absorbed into fused
scalar_tensor_tensor ops.

Host side only transposes/shards numpy arrays; all compute is on device.
"""

import numpy as np

import concourse.bass as bass
import concourse.tile as tile
from concourse import bacc, mybir
from concourse.bass_utils import run_bass_kernel_spmd

# ---- problem constants (hardcoded; kernel.py must be self-contained) ----
B = 65536
N_CORES = 8
ROWS = B // N_CORES          # 8192 rows per core
NB = 256                     # batch tile (free dim)
NT = ROWS // NB              # 16 tiles per core

GEN = 200
CON = 128
CO = 4
LAT = 64
CIE = 128                    # CI_ENC_DIM
EXT = 16
CLIP = 5.0
GEN_IN = EXT + CO            # 20
CON_IN = 2 * CIE + LAT       # 320
STATE = 420

F32 = mybir.dt.float32
F32R = mybir.dt.float32r
BF16 = mybir.dt.bfloat16
# dtype of the gate elementwise chain (tanh outputs, d/e/blend temps).
# bf16 halves some DVE op costs on HW but adds ~3e-3 relative error;
# fp32 keeps the kernel at ~1.8e-4 (fp32r matmul precision).
GATE_DT = F32
AF = mybir.ActivationFunctionType
ALU = mybir.AluOpType


# packed-weight column layout: name -> (rows, cols, col_offset)
_WCOLS = {}
_off = 0
for _nm, _p, _f in (
    ("cwA", 128, 384), ("cwB", 128, 384), ("cwC", 65, 384), ("cwH", 128, 384),
    ("cbHN", 1, 128), ("gwI", 21, 600), ("gwHA", 128, 600), ("gwHB", 73, 600),
    ("coW", 128, 8), ("coB", 1, 8), ("coBm", 4, 1), ("coBv", 4, 1),
    ("facWA", 128, 64), ("facWB", 72, 64),
    ("ones", 1, 512),
):
    _WCOLS[_nm] = (_p, _f, _off)
    _off += _f
WPACK_COLS = _off


def _f(ap):

    """View an fp32r AP as plain fp32 for DVE/ACT/DMA use."""
    return ap.bitcast(F32)


def build_decoder(nc: bass.Bass, tc: tile.TileContext, ctx, ins: dict, outs: dict,
                  rows: int = ROWS, nb: int = NB):
    """Emit the per-core program. `ins`/`outs` map name -> DRAM AP.

    Super-tiles of 2*nb rows are loaded/stored with packed DMAs; compute
    runs on nb-wide subtiles. All DRAM layouts are host-packed tile-major.
    """
    NB = nb           # noqa: N806
    NB2 = 2 * nb      # noqa: N806 — super-tile width
    NST = rows // NB2  # noqa: N806

    wp = ctx.enter_context(tc.tile_pool(name="wp", bufs=1))
    lp = ctx.enter_context(tc.tile_pool(name="lp", bufs=4))
    gp = ctx.enter_context(tc.tile_pool(name="gp", bufs=2))
    op = ctx.enter_context(tc.tile_pool(name="op", bufs=4))
    pp = ctx.enter_context(tc.tile_pool(name="pp", bufs=8, space="PSUM"))

    # ---- persistent weights in SBUF: one packed tile, one DMA ----
    wsb = wp.tile([128, WPACK_COLS], F32R, name="wsb")
    nc.sync.dma_start(wsb[:], ins["wpack"][:])

    def wv(name):
        p, f, c0 = _WCOLS[name]
        return wsb[0:p, c0:c0 + f]

    cwA, cwB, cwC, cwH = wv("cwA"), wv("cwB"), wv("cwC"), wv("cwH")
    cbHN, gwI, gwHA, gwHB = wv("cbHN"), wv("gwI"), wv("gwHA"), wv("gwHB")
    coW, coB, facWA, facWB = wv("coW"), wv("coB"), wv("facWA"), wv("facWB")
    coBm, coBv = _f(wv("coBm")), _f(wv("coBv"))
    ones = wv("ones")

    mm = nc.tensor.matmul
    CH = 2  # super-tiles per pipeline chunk

    def stage_load(st):
        c2 = slice(st * NB2, (st + 1) * NB2)
        c4 = slice(st * 2 * NB2, (st + 1) * 2 * NB2)
        grp1 = lp.tile([128, 2 * NB2], F32R, name="grp1")   # [ci0 | ci1]
        nc.sync.dma_start(grp1[:], ins["grp1"][:, c4])
        grp2 = lp.tile([128, 2 * NB2], F32R, name="grp2")   # [con_s | gen0]
        nc.sync.dma_start(grp2[:], ins["grp2"][:, c4])
        grp3 = lp.tile([73, 2 * NB2], F32R, name="grp3")    # [gen1 | facp]
        nc.sync.dma_start(grp3[:], ins["grp3"][:, c4])
        gin = gp.tile([68, NB2], F32R, name="gin", bufs=4)
        nc.sync.dma_start(gin[4:68, :], ins["ginb3"][:, c2])
        epsv = gp.tile([CO, NB2], F32, name="epsv", bufs=4)
        nc.sync.dma_start(epsv[:], _f(ins["ginb3"][28:32, c2]))
        og1 = op.tile([128, 2 * NB2], F32R, name="og1")     # [genpA | conp]
        og2 = op.tile([72, NB2], F32R, name="og2")          # genpB
        fct = op.tile([64, NB2], F32, name="fct")           # factor
        return dict(st=st, c2=c2, c4=c4, grp1=grp1, grp2=grp2, grp3=grp3,
                    gin=gin, epsv=epsv, og1=og1, og2=og2, fct=fct)

    # Merged layouts (all blocks in subtile order s0|s1, NB wide each):
    #   p_crz [128, 4NB] = [r_s0 | r_s1 | z_s0 | z_s1]
    #   p_cn  [128, 4NB] = [i_s0 | i_s1 | h_s0 | h_s1]
    # Elementwise then runs once per super-tile at FD = NB2.

    def stage_con_a(io):
        grp1, grp2, grp3 = io["grp1"], io["grp2"], io["grp3"]
        p_cr = pp.tile([128, NB2], F32, name="p_cr", tag="pp")
        p_cz = pp.tile([128, NB2], F32, name="p_cz", tag="pp")
        p_ci = pp.tile([128, NB2], F32, name="p_ci", tag="pp")
        p_ch = pp.tile([128, NB2], F32, name="p_ch", tag="pp")
        for s in range(2):
            cs = slice(s * NB, (s + 1) * NB)
            ci0 = grp1[0:128, s * NB:(s + 1) * NB]
            ci1 = grp1[0:128, NB2 + s * NB:NB2 + (s + 1) * NB]
            con_s = grp2[0:128, s * NB:(s + 1) * NB]
            facp = grp3[0:65, NB2 + s * NB:NB2 + (s + 1) * NB]
            for dst, c0 in ((p_cr, 0), (p_cz, 128)):
                mm(dst[:, cs], cwA[:, c0:c0 + 128], ci0, start=True, stop=False)
                mm(dst[:, cs], cwB[:, c0:c0 + 128], ci1, start=False, stop=False)
                mm(dst[:, cs], cwC[:, c0:c0 + 128], facp, start=False, stop=False)
                mm(dst[:, cs], cwH[:, c0:c0 + 128], con_s, start=False, stop=True)
            mm(p_ci[:, cs], cwA[:, 256:384], ci0, start=True, stop=False)
            mm(p_ci[:, cs], cwB[:, 256:384], ci1, start=False, stop=False)
            mm(p_ci[:, cs], cwC[:, 256:384], facp, start=False, stop=True)
            mm(p_ch[:, cs], cwH[:, 256:384], con_s, start=True, stop=False)
            mm(p_ch[:, cs], cbHN[:], ones[:, 0:NB], start=False, stop=True)

        t_crz = gp.tile([128, 2 * NB2], GATE_DT, name="t_crz")
        nc.scalar.activation(t_crz[:, 0:NB2], p_cr[:], AF.Tanh, scale=0.5)
        nc.scalar.activation(t_crz[:, NB2:2 * NB2], p_cz[:], AF.Tanh, scale=0.5)
        tp_c = gp.tile([128, NB2], F32, name="tp_c")
        nc.vector.scalar_tensor_tensor(  # (1+tanh_r)*h_n == 2*r*h_n
            tp_c[:], t_crz[:, 0:NB2], 1.0, p_ch[:],
            op0=ALU.add, op1=ALU.mult)
        u_c = gp.tile([128, NB2], F32, name="u_c")
        nc.vector.scalar_tensor_tensor(  # 0.5*(2*r*h_n) + i_n
            u_c[:], tp_c[:], 0.5, p_ci[:], op0=ALU.mult, op1=ALU.add)
        io["t_crz"], io["u_c"] = t_crz, u_c

    def stage_con_b(io):
        t_crz, u_c = io.pop("t_crz"), io.pop("u_c")
        n_c = gp.tile([128, NB2], GATE_DT, name="n_c")
        nc.scalar.activation(n_c[:], u_c[:], AF.Tanh)
        d_c = gp.tile([128, NB2], GATE_DT, name="d_c")
        nc.gpsimd.tensor_sub(d_c[:], _f(io["grp2"][0:128, 0:NB2]), n_c[:])
        e_c = gp.tile([128, NB2], GATE_DT, name="e_c")
        nc.vector.scalar_tensor_tensor(  # (1+tanh_z)*(h-n)
            e_c[:], t_crz[:, NB2:2 * NB2], 1.0, d_c[:], op0=ALU.add, op1=ALU.mult)
        cpre = gp.tile([128, NB2], GATE_DT, name="cpre")
        nc.vector.scalar_tensor_tensor(  # n + 0.5*e
            cpre[:], e_c[:], 0.5, n_c[:], op0=ALU.mult, op1=ALU.add)
        nc.gpsimd.tensor_scalar(  # clip both subtiles into packed output
            io["og1"][0:128, NB2:2 * NB2], cpre[:], CLIP, -CLIP,
            op0=ALU.min, op1=ALU.max)

    def stage_co(io):
        gin = io["gin"]
        p_cm = pp.tile([CO, NB2], F32, name="p_cm", tag="pp")
        p_cv = pp.tile([CO, NB2], F32, name="p_cv", tag="pp")
        for s in range(2):
            conp = io["og1"][0:128, NB2 + s * NB:NB2 + (s + 1) * NB]
            cs = slice(s * NB, (s + 1) * NB)
            mm(p_cm[:, cs], coW[:, 0:CO], conp, start=True, stop=True)
            mm(p_cv[:, cs], coW[:, CO:2 * CO], conp, start=True, stop=True)
        # biases fold into the ACT affine (per-partition = per-gate here)
        stdt = gp.tile([CO, NB2], F32, name="stdt")
        nc.scalar.activation(stdt[:], p_cv[:], AF.Exp, scale=0.5, bias=coBv)
        q_co = gp.tile([CO, NB2], F32, name="q_co")
        nc.gpsimd.tensor_mul(q_co[:], stdt[:], io["epsv"][0:CO, :])  # std*eps
        nc.scalar.activation(gin[64:68, :], p_cm[:], AF.Identity,
                             bias=coBm)                              # co_mean
        nc.vector.tensor_copy(gin[32:36, :], stdt[:])                # co_std out
        # con_out = (std*eps + b_mean) + mean_raw   (mean from PSUM)
        nc.vector.scalar_tensor_tensor(
            gin[0:CO, :], q_co[:], coBm, p_cm[:], op0=ALU.add, op1=ALU.add)

    def stage_gen_a(io):
        grp2, grp3, gin = io["grp2"], io["grp3"], io["gin"]
        for (msz, m0) in ((128, 0), (72, 128)):
            p_gr = pp.tile([msz, NB2], F32, name=f"p_gr{m0}", tag="pp")
            p_gz = pp.tile([msz, NB2], F32, name=f"p_gz{m0}", tag="pp")
            p_gi = pp.tile([msz, NB2], F32, name=f"p_gi{m0}", tag="pp")
            p_gh = pp.tile([msz, NB2], F32, name=f"p_gh{m0}", tag="pp")
            for s in range(2):
                cs = slice(s * NB, (s + 1) * NB)
                g_in = gin[0:GEN_IN + 1, s * NB:(s + 1) * NB]
                gen0 = grp2[0:128, NB2 + s * NB:NB2 + (s + 1) * NB]
                gen1 = grp3[0:73, s * NB:(s + 1) * NB]
                for dst, c0 in ((p_gr, m0), (p_gz, 200 + m0)):
                    mm(dst[:, cs], gwI[:, c0:c0 + msz], g_in,
                       start=True, stop=False)
                    mm(dst[:, cs], gwHA[:, c0:c0 + msz], gen0,
                       start=False, stop=False)
                    mm(dst[:, cs], gwHB[:, c0:c0 + msz], gen1,
                       start=False, stop=True)
                mm(p_gi[:, cs], gwI[:, 400 + m0:400 + m0 + msz], g_in,
                   start=True, stop=True)
                mm(p_gh[:, cs], gwHA[:, 400 + m0:400 + m0 + msz], gen0,
                   start=True, stop=False)
                mm(p_gh[:, cs], gwHB[:, 400 + m0:400 + m0 + msz], gen1,
                   start=False, stop=True)

            t_grz = gp.tile([msz, 2 * NB2], GATE_DT, name=f"t_grz{m0}", tag="t_grz")
            nc.scalar.activation(t_grz[:, 0:NB2], p_gr[:], AF.Tanh, scale=0.5)
            nc.scalar.activation(t_grz[:, NB2:2 * NB2], p_gz[:], AF.Tanh, scale=0.5)
            tp_g = gp.tile([msz, NB2], F32, name=f"tp_g{m0}", tag="tp_g")
            nc.vector.scalar_tensor_tensor(
                tp_g[:], t_grz[:, 0:NB2], 1.0, p_gh[:],
                op0=ALU.add, op1=ALU.mult)
            u_g = gp.tile([msz, NB2], F32, name=f"u_g{m0}", tag="u_g")
            nc.vector.scalar_tensor_tensor(
                u_g[:], tp_g[:], 0.5, p_gi[:], op0=ALU.mult, op1=ALU.add)
            io[f"t_grz{m0}"], io[f"u_g{m0}"] = t_grz, u_g

    def stage_gen_b(io):
        for (msz, m0, h_blk, outp) in (
            (128, 0, io["grp2"][0:128, NB2:2 * NB2], io["og1"][0:128, 0:NB2]),
            (72, 128, io["grp3"][0:72, 0:NB2], io["og2"][0:72, 0:NB2]),
        ):
            t_grz, u_g = io.pop(f"t_grz{m0}"), io.pop(f"u_g{m0}")
            n_g = gp.tile([msz, NB2], GATE_DT, name=f"n_g{m0}", tag="n_g")
            nc.scalar.activation(n_g[:], u_g[:], AF.Tanh)
            d_g = gp.tile([msz, NB2], GATE_DT, name=f"d_g{m0}", tag="d_g")
            nc.gpsimd.tensor_sub(d_g[:], _f(h_blk), n_g[:])
            e_g = gp.tile([msz, NB2], GATE_DT, name=f"e_g{m0}", tag="e_g")
            nc.vector.scalar_tensor_tensor(
                e_g[:], t_grz[:, NB2:2 * NB2], 1.0, d_g[:],
                op0=ALU.add, op1=ALU.mult)
            gpre = gp.tile([msz, NB2], GATE_DT, name=f"gpre{m0}", tag="gpre")
            nc.vector.scalar_tensor_tensor(
                gpre[:], e_g[:], 0.5, n_g[:], op0=ALU.mult, op1=ALU.add)
            nc.gpsimd.tensor_scalar(
                outp, gpre[:], CLIP, -CLIP, op0=ALU.min, op1=ALU.max)

    def stage_fac(io):
        p_f = pp.tile([LAT, NB2], F32, name="p_f", tag="pp")
        for s in range(2):
            cs = slice(s * NB, (s + 1) * NB)
            mm(p_f[:, cs], facWA[:], io["og1"][0:128, s * NB:(s + 1) * NB],
               start=True, stop=False)
            mm(p_f[:, cs], facWB[:], io["og2"][0:72, s * NB:(s + 1) * NB],
               start=False, stop=True)
        nc.scalar.copy(io["fct"][:], p_f[:])

    def stage_store(io):
        nc.sync.dma_start(outs["og1"][:, io["c4"]], _f(io["og1"][:]))
        nc.sync.dma_start(outs["og2"][:, io["c2"]], _f(io["og2"][:]))
        nc.sync.dma_start(outs["fct"][:, io["c2"]], io["fct"][:])
        nc.sync.dma_start(outs["ginout"][:, io["c2"]], _f(io["gin"][0:68, :]))

    assert NST % CH == 0
    # Chunk-level software pipeline: chunk k's controller matmuls fill the
    # PE queue while chunk k-1's gate->co->sample chain drains, and chunk
    # k-1's generator matmuls hide chunk k's controller chain.
    prev = None
    for ch in range(NST // CH):
        ios = [stage_load(ch * CH + i) for i in range(CH)]
        if prev is not None:
            for io in prev:
                stage_gen_a(io)
        for io in ios:
            stage_con_a(io)
        if prev is not None:
            for io in prev:
                stage_gen_b(io)
            for io in prev:
                stage_fac(io)
            for io in prev:
                stage_store(io)
        for io in ios:
            stage_con_b(io)
        for io in ios:
            stage_co(io)
        prev = ios
    for io in prev:
        stage_gen_a(io)
    for io in prev:
        stage_gen_b(io)
    for io in prev:
        stage_fac(io)
    for io in prev:
        stage_store(io)


def _weight_arrays(gen_w_ih, gen_w_hh, gen_b_ih, gen_b_hh,
                   con_w_ih, con_w_hh, con_b_ih, con_b_hh, co_w, co_b, fac_w):
    """Host-side weight prep: transpose + bias-row augmentation."""
    f = np.float32
    cw = np.ascontiguousarray(con_w_ih.T, dtype=f)      # [320, 384]
    cbias = con_b_ih.astype(f).copy()
    cbias[:256] += con_b_hh[:256].astype(f)             # rz combined; n = b_ih only
    cwC = np.concatenate([cw[256:320], cbias[None, :]], axis=0)
    gw = np.ascontiguousarray(gen_w_ih.T, dtype=f)      # [20, 600]
    gbias = gen_b_ih.astype(f).copy()
    gbias[:400] += gen_b_hh[:400].astype(f)
    gwI = np.concatenate([gw, gbias[None, :]], axis=0)  # [21, 600]
    gh = np.ascontiguousarray(gen_w_hh.T, dtype=f)      # [200, 600]
    ghb = np.zeros((1, 600), dtype=f)
    ghb[0, 400:] = gen_b_hh[400:]
    gwHB = np.concatenate([gh[128:200], ghb], axis=0)   # [73, 600]
    nrm = np.maximum(np.linalg.norm(fac_w.astype(np.float64), axis=1,
                                    keepdims=True), 1e-12)
    facn = np.ascontiguousarray((fac_w / nrm).T, dtype=f)  # [200, 64]
    parts = {
        "cwA": cw[0:128], "cwB": cw[128:256], "cwC": cwC,
        "cwH": np.ascontiguousarray(con_w_hh.T, dtype=f),
        "cbHN": con_b_hh[256:384].astype(f).reshape(1, 128),
        "gwI": gwI, "gwHA": gh[0:128], "gwHB": gwHB,
        "coW": np.ascontiguousarray(co_w.T, dtype=f),
        "coB": co_b.astype(f).reshape(1, 8),
        "coBm": co_b[0:4].astype(f).reshape(4, 1),
        "coBv": (0.5 * co_b[4:8]).astype(f).reshape(4, 1),
        "facWA": facn[0:128], "facWB": facn[128:200],
        "ones": np.ones((1, 512), dtype=f),
    }
    wpack = np.zeros((128, WPACK_COLS), dtype=f)
    for nm, (p, fc, c0) in _WCOLS.items():
        wpack[0:p, c0:c0 + fc] = parts[nm]
    return {"wpack": wpack}


_CACHED = {}


def _build_nc(rows=ROWS, nb=NB):
    if (rows, nb) in _CACHED:
        return _CACHED[(rows, nb)]
    from contextlib import ExitStack

    nc = bacc.Bacc("TRN2", target_bir_lowering=False, debug=False,
                   num_devices=N_CORES)
    names_in = {
        "grp1": [128, 2 * rows], "grp2": [128, 2 * rows],
        "grp3": [73, 2 * rows], "ginb3": [64, rows],
        "wpack": [128, WPACK_COLS],
    }
    ins = {k: nc.dram_tensor(k, v, F32R, kind="ExternalInput").ap()
           for k, v in names_in.items()}
    outs = {
        "og1": nc.dram_tensor("og1", [128, 2 * rows], F32,
                              kind="ExternalOutput").ap(),
        "og2": nc.dram_tensor("og2", [72, rows], F32,
                              kind="ExternalOutput").ap(),
        "fct": nc.dram_tensor("fct", [64, rows], F32,
                              kind="ExternalOutput").ap(),
        "ginout": nc.dram_tensor("ginout", [68, rows], F32,
                                 kind="ExternalOutput").ap(),
    }
    with tile.TileContext(nc) as tc:
        with ExitStack() as ctx:
            build_decoder(nc, tc, ctx, ins, outs, rows=rows, nb=nb)
    nc.compile()
    _CACHED[(rows, nb)] = nc
    return nc


def pack_inputs(x, h0, eps, rows, nb=NB):
    """Host-side tile-major packing of one core's activations."""
    f = np.float32
    nb2 = 2 * nb
    nst = rows // nb2
    one = np.ones((1, rows), dtype=f)

    def inter(a, b):
        # [p, rows] x2 -> [p, 2*rows] with per-super-tile [a_block | b_block]
        p = a.shape[0]
        out = np.empty((p, 2 * rows), dtype=f)
        av = a.reshape(p, nst, nb2)
        bv = b.reshape(p, nst, nb2)
        ov = out.reshape(p, nst, 2, nb2)
        ov[:, :, 0, :] = av
        ov[:, :, 1, :] = bv
        return out

    xT = x.T  # [272, rows]
    grp1 = inter(np.ascontiguousarray(xT[0:128]), np.ascontiguousarray(xT[128:256]))
    grp2 = inter(np.ascontiguousarray(h0[:, 200:328].T),
                 np.ascontiguousarray(h0[:, 0:128].T))
    gen1 = np.concatenate([h0[:, 128:200].T, one], axis=0)          # [73, rows]
    facp = np.concatenate([h0[:, 356:420].T, one,
                           np.zeros((8, rows), dtype=f)], axis=0)   # [73, rows]
    grp3 = inter(np.ascontiguousarray(gen1), facp)
    ginb3 = np.concatenate([
        x[:, 256:272].T, one, np.zeros((11, rows), dtype=f), eps.T,
        np.zeros((32, rows), dtype=f),
    ], axis=0)                                                       # [64, rows]
    return {"grp1": grp1, "grp2": grp2, "grp3": grp3,
            "ginb3": np.ascontiguousarray(ginb3)}


def unpack_outputs(res, rows, nb=NB):
    """Invert the packed og1/og2/ginout layouts into [rows, 420]."""
    nb2 = 2 * nb
    nst = rows // nb2
    out = np.empty((rows, STATE), dtype=np.float32)
    og1 = res["og1"].reshape(128, nst, 2, nb2)   # [genpA | conp]
    genpA = og1[:, :, 0, :].reshape(128, rows)
    conp = og1[:, :, 1, :].reshape(128, rows)
    gin = res["ginout"]                          # [68, rows]
    out[:, 0:128] = genpA.T
    out[:, 128:200] = res["og2"].T
    out[:, 200:328] = conp.T
    out[:, 328:332] = gin[64:68].T
    out[:, 332:336] = gin[32:36].T
    out[:, 336:356] = gin[0:20].T
    out[:, 356:420] = res["fct"].T
    return out


def kernel(x, h0, eps, gen_w_ih, gen_w_hh, gen_b_ih, gen_b_hh,
           con_w_ih, con_w_hh, con_b_ih, con_b_hh, co_w, co_b, fac_w,
           **run_kwargs):
    x = np.asarray(x, dtype=np.float32)
    h0 = np.asarray(h0, dtype=np.float32)
    eps = np.asarray(eps, dtype=np.float32)
    w = _weight_arrays(gen_w_ih, gen_w_hh, gen_b_ih, gen_b_hh,
                       con_w_ih, con_w_hh, con_b_ih, con_b_hh,
                       co_w, co_b, fac_w)
    nc = _build_nc()

    in_maps = []
    for c in range(N_CORES):
        r0, r1 = c * ROWS, (c + 1) * ROWS
        m = dict(w)
        m.update(pack_inputs(x[r0:r1], h0[r0:r1], eps[r0:r1], ROWS))
        in_maps.append(m)

    res = run_bass_kernel_spmd(nc, in_maps, core_ids=list(range(N_CORES)),
                               **run_kwargs)
    out = np.empty((B, STATE), dtype=np.float32)
    for c in range(N_CORES):
        out[c * ROWS:(c + 1) * ROWS] = unpack_outputs(res.results[c], ROWS)
    if run_kwargs:
        return out, res
    return out



# revision 27
# speedup vs baseline: 1.5133x; 1.5133x over previous
"""Trainium2 Bass kernel for nn_DecoderCell (LFADS decoder cell), v2.

Strategy: pure data parallel over 8 NeuronCores (8192 batch rows each).
On-chip layout is transposed ([feature, batch]); batch rides the free dim in
512-wide compute tiles (1024-wide DMA tiles). All activation I/O is bf16
(halves HBM traffic); matmuls are bf16 (full PE rate at any free size).
K-chunks are packed so each GRU needs the minimum number of matmul
instructions (con 12, gen 14 per 512 cols). The co/fac linears run
batch-major (activations stationary) so their free dim is the tiny feature
count. Sigmoid is synthesized from tanh (one ACT table set: Exp+Tanh);
biases ride ACT bias APs / conditional TS-adds (zero for this problem).

Host side only transposes/casts/shards numpy arrays; all compute on device.
"""

import numpy as np

import concourse.bass as bass
import concourse.tile as tile
from concourse import bacc, mybir
from concourse.bass_utils import run_bass_kernel_spmd

# ---- problem constants (hardcoded; kernel.py must be self-contained) ----
B = 65536
N_CORES = 8
ROWS = B // N_CORES          # 8192 rows per core
F = 512                      # batch tile (free dim) per compute step
NST = ROWS // F              # 16 compute tiles per core
F2 = 2 * F                   # DMA/store tile width
NG = ROWS // F2              # 8 DMA groups per core

GEN = 200
CON = 128
CO = 4
LAT = 64
CIE = 128
EXT = 16
CLIP = 5.0
STATE = 420

F32 = mybir.dt.float32
BF16 = mybir.dt.bfloat16
NPBF = mybir.dt.np(BF16)
AF = mybir.ActivationFunctionType
ALU = mybir.AluOpType

# weight pack column layout (bf16): name -> (rows, cols, col_offset)
_WCOLS = {}
_off = 0
for _nm, _p, _c in (
    ("Wc1", 128, 384), ("Wc2", 128, 384), ("Wc4", 64, 384),
    ("Wc3rz", 128, 256), ("Wc3n", 128, 128),
    ("Wg1", 128, 600), ("Wg2", 92, 600), ("Wgx", 20, 200),
    ("Wco", 128, 8), ("Wf1", 128, 64), ("Wf2", 72, 64),
    ("Ident", 128, 128),
):
    _WCOLS[_nm] = (_p, _c, _off)
    _off += _c
WPACK_COLS = _off

# bias vector pack (f32): name -> column
_BCOLS = {nm: i for i, nm in enumerate(
    ("b_cr05", "b_cz05", "b_cin", "b_chn",
     "b_gr05_0", "b_gr05_1", "b_gz05_0", "b_gz05_1",
     "b_gin_0", "b_gin_1", "b_ghn_0", "b_ghn_1",
     "b_m", "b_v05"))}
NBCOLS = len(_BCOLS)

# m1 packed input rows: [fac 64 | hg1 72 | ext 16 | eps 4]
M1_ROWS = 64 + 72 + 16 + 4   # 156

# Matmul operand bases must be 0/32/64 (32/64 with limited spans), engine-op
# bases 0/32/64/96. g2t holds the 92-row gen rz K-chunk; xt holds the 20-row
# x block (con_out engine-written at base 0), DMA-copied into g2t[72:76].
#   g2t: 0:72 hg1 | 72:76 con_out copy | 76:92 ext
G2_ROWS = 92


def build_decoder(nc: bass.Bass, tc: tile.TileContext, ctx, ins, outs,
                  has_bias: bool):
    wp = ctx.enter_context(tc.tile_pool(name="wp", bufs=1))
    lp = ctx.enter_context(tc.tile_pool(name="lp", bufs=3))
    gp = ctx.enter_context(tc.tile_pool(name="gp", bufs=4))
    op = ctx.enter_context(tc.tile_pool(name="op", bufs=3))
    ppb = ctx.enter_context(tc.tile_pool(name="ppb", bufs=2, space="PSUM"))
    pps = ctx.enter_context(tc.tile_pool(name="pps", bufs=3, space="PSUM"))
    ppq = ctx.enter_context(tc.tile_pool(name="ppq", bufs=1, space="PSUM"))

    wsb = wp.tile([128, WPACK_COLS], BF16, name="wsb")
    nc.sync.dma_start(wsb[:], ins["wpack"][:])
    bvt = wp.tile([128, NBCOLS], F32, name="bvt")
    nc.sync.dma_start(bvt[:], ins["bvec"][:])

    def wv(name):
        p, c, c0 = _WCOLS[name]
        return wsb[0:p, c0:c0 + c]

    def bv(name, p=128):
        return bvt[0:p, _BCOLS[name]:_BCOLS[name] + 1]

    Wc1, Wc2, Wc4 = wv("Wc1"), wv("Wc2"), wv("Wc4")
    Wc3rz, Wc3n = wv("Wc3rz"), wv("Wc3n")
    Wg1, Wg2, Wco = wv("Wg1"), wv("Wg2"), wv("Wco")
    Wgx = wv("Wgx")
    Ident = wv("Ident")
    Wf1, Wf2 = wv("Wf1"), wv("Wf2")

    mm = nc.tensor.matmul

    # ---------------- per-group (2 tiles) load ----------------
    def stage_load(g):
        cg = slice(g * F2, (g + 1) * F2)
        cct = lp.tile([128, 4 * F2], BF16, name="cct", tag="cct")
        nc.sync.dma_start(cct[:], ins["cc"][:, g * 4 * F2:(g + 1) * 4 * F2])
        c1t = cct[:, 0:F2]
        c2t = cct[:, F2:2 * F2]
        c3t = cct[:, 2 * F2:3 * F2]
        g1t = cct[:, 3 * F2:4 * F2]
        c4t = lp.tile([64, F2], BF16, name="c4t", tag="c4t")
        nc.sync.dma_start(c4t[:], ins["m1"][0:64, cg])
        g2t = lp.tile([G2_ROWS, F2], BF16, name="g2t", tag="g2t")
        nc.sync.dma_start(g2t[0:72, :], ins["m1"][64:136, cg])
        nc.sync.dma_start(g2t[76:92, :], ins["m1"][136:152, cg])
        xt = lp.tile([20, F2], BF16, name="xt", tag="xt")
        nc.sync.dma_start(xt[4:20, :], ins["m1"][136:152, cg])
        epst = lp.tile([4, F2], BF16, name="epst", tag="epst")
        nc.sync.dma_start(epst[:], ins["m1"][152:156, cg])
        oga = op.tile([128, 2 * F2], BF16, name="oga", tag="oga")
        og1 = oga[:, 0:F2]
        ogc = oga[:, F2:2 * F2]
        og2 = op.tile([72, F2], BF16, name="og2", tag="og2")
        fct = op.tile([128, F], BF16, name="fct", tag="fct")
        return dict(g=g, cg=cg, c1t=c1t, c2t=c2t, c3t=c3t, g1t=g1t, c4t=c4t,
                    g2t=g2t, xt=xt, epst=epst, oga=oga, ogc=ogc, og1=og1,
                    og2=og2, fct=fct)

    # ------------- controller GRU: matmuls + first eltwise -------------
    def stage_con_a(io, s):
        cs = slice(s * F, (s + 1) * F)
        c1 = io["c1t"][:, cs]
        c2 = io["c2t"][:, cs]
        c3 = io["c3t"][:, cs]
        c4 = io["c4t"][:, cs]
        a_crz = ppb.tile([128, F2], F32, name="a_crz", tag="rz")
        for d, c0 in ((slice(0, F), 0), (slice(F, F2), 128)):
            mm(a_crz[:, d], Wc1[:, c0:c0 + 128], c1, start=True, stop=False)
            mm(a_crz[:, d], Wc2[:, c0:c0 + 128], c2, start=False, stop=False)
            mm(a_crz[:, d], Wc4[:, c0:c0 + 128], c4, start=False, stop=False)
            mm(a_crz[:, d], Wc3rz[:, c0:c0 + 128], c3, start=False, stop=True)
        a_ci = pps.tile([128, F], F32, name="a_ci", tag="sm")
        mm(a_ci[:], Wc1[:, 256:384], c1, start=True, stop=False)
        mm(a_ci[:], Wc2[:, 256:384], c2, start=False, stop=False)
        mm(a_ci[:], Wc4[:, 256:384], c4, start=False, stop=False)
        a_ch = pps.tile([128, F], F32, name="a_ch", tag="sm")
        mm(a_ch[:], Wc3n, c3, start=True, stop=True)

        if has_bias:
            nc.vector.tensor_scalar(a_crz[:, 0:F], a_crz[:, 0:F],
                                    bv("b_cr05"), None, op0=ALU.add)
            nc.vector.tensor_scalar(a_crz[:, F:F2], a_crz[:, F:F2],
                                    bv("b_cz05"), None, op0=ALU.add)
            nc.vector.tensor_scalar(a_ci[:], a_ci[:], bv("b_cin"), None,
                                    op0=ALU.add)
            nc.vector.tensor_scalar(a_ch[:], a_ch[:], bv("b_chn"), None,
                                    op0=ALU.add)
        t_crz = gp.tile([128, F2], BF16, name="t_crz", tag="t_crz")
        nc.scalar.activation(t_crz[:], a_crz[:], AF.Tanh, scale=0.5)
        # sigma = (tanh + 1)/2 via TS (4x DVE mode on bf16)
        w_crz = gp.tile([128, F2], BF16, name="w_crz", tag="w_crz")
        nc.vector.tensor_scalar(w_crz[:], t_crz[:], 1.0, 0.5, op0=ALU.add,
                                op1=ALU.mult)
        tp_c = gp.tile([128, F], BF16, name="tp_c", tag="tp_c")
        nc.vector.tensor_tensor(tp_c[:], w_crz[:, 0:F], a_ch[:], op=ALU.mult)
        # u = a_i + r*h_n via identity-matmul accumulation onto a_ci
        mm(a_ci[:], Ident, tp_c[:], start=False, stop=True)
        io[f"w_crz{s}"], io[f"a_ci{s}"] = w_crz, a_ci

    def stage_con_a2(io, s):
        a_ci = io.pop(f"a_ci{s}")
        n_c = gp.tile([128, F], BF16, name="n_c", tag="n_c")
        nc.scalar.activation(n_c[:], a_ci[:], AF.Tanh)
        io[f"n_c{s}"] = n_c

    # ------------- controller GRU: second eltwise + clip -------------
    def stage_con_b(io, s):
        cs = slice(s * F, (s + 1) * F)
        w_crz, n_c = io.pop(f"w_crz{s}"), io.pop(f"n_c{s}")
        d_c = gp.tile([128, F], BF16, name="d_c", tag="d_c")
        nc.vector.tensor_tensor(d_c[:], io["c3t"][:, cs], n_c[:],
                                op=ALU.subtract)
        e_c = gp.tile([128, F], BF16, name="e_c", tag="e_c")
        nc.vector.tensor_tensor(e_c[:], w_crz[:, F:F2], d_c[:], op=ALU.mult)
        hp_c = gp.tile([128, F], BF16, name="hp_c", tag="hp_c")
        nc.vector.tensor_tensor(hp_c[:], e_c[:], n_c[:], op=ALU.add)
        nc.vector.tensor_scalar(io["ogc"][:, cs], hp_c[:], CLIP, -CLIP,
                                op0=ALU.min, op1=ALU.max)

    # ------------- controller output sample (feature-major) -------------
    def stage_co(io, s):
        cs = slice(s * F, (s + 1) * F)
        g2t = io["g2t"]
        p_co = ppq.tile([36, F], F32, name="p_co", tag="sq")
        p_cm = p_co[0:4, :]
        p_cv = p_co[32:36, :]
        mm(p_cm, Wco[:, 0:4], io["ogc"][:, cs], start=True, stop=True)
        mm(p_cv, Wco[:, 4:8], io["ogc"][:, cs], start=True, stop=True)
        # cot cols: [mean | std | q], all at partition base 0
        cot = gp.tile([4, 3 * F], BF16, name="cot", tag="cot")
        # std = exp(0.5*logvar + 0.5*b_v); bias AP is free
        nc.scalar.activation(cot[:, F:F2], p_cv, AF.Exp,
                             scale=0.5, bias=bv("b_v05", 4))
        # mean = p_cm + b_m
        nc.vector.tensor_scalar(cot[:, 0:F], p_cm, bv("b_m", 4),
                                None, op0=ALU.add)
        nc.gpsimd.tensor_tensor(cot[:, F2:3 * F], cot[:, F:F2],
                                io["epst"][:, cs], op=ALU.mult)
        # con_out = mean + std*eps (mean already biased, bf16-rounded)
        nc.gpsimd.tensor_tensor(io["xt"][0:4, cs], cot[:, F2:3 * F],
                                cot[:, 0:F], op=ALU.add)
        io[f"cot{s}"] = cot

    # ------------- generator GRU: matmuls + first eltwise -------------
    def stage_gen_a(io, s):
        cs = slice(s * F, (s + 1) * F)
        g1 = io["g1t"][:, cs]
        g2k = io["g2t"][0:92, cs]
        g2h = io["g2t"][0:72, cs]
        g2x = io["xt"][0:20, cs]
        a_grz0 = ppb.tile([128, F2], F32, name="a_grz0", tag="rz")
        a_grz1 = ppb.tile([72, F2], F32, name="a_grz1", tag="rz")
        for d, c0 in ((slice(0, F), 0), (slice(F, F2), 200)):
            mm(a_grz0[:, d], Wg1[:, c0:c0 + 128], g1, start=True, stop=False)
            mm(a_grz0[:, d], Wg2[:, c0:c0 + 128], g2k, start=False, stop=True)
            mm(a_grz1[:, d], Wg1[:, c0 + 128:c0 + 200], g1,
               start=True, stop=False)
            mm(a_grz1[:, d], Wg2[:, c0 + 128:c0 + 200], g2k,
               start=False, stop=True)
        a_gi0 = pps.tile([128, F], F32, name="a_gi0", tag="sm")
        mm(a_gi0[:], Wgx[:, 0:128], g2x, start=True, stop=False)
        a_gi1 = pps.tile([72, F], F32, name="a_gi1", tag="sm")
        mm(a_gi1[:], Wgx[:, 128:200], g2x, start=True, stop=False)
        a_gh = ppb.tile([128, F2], F32, name="a_gh", tag="rz")
        mm(a_gh[:, 0:F], Wg1[:, 400:528], g1, start=True, stop=False)
        mm(a_gh[:, 0:F], Wg2[0:72, 400:528], g2h, start=False, stop=True)
        mm(a_gh[0:72, F:F2], Wg1[:, 528:600], g1, start=True, stop=False)
        mm(a_gh[0:72, F:F2], Wg2[0:72, 528:600], g2h, start=False, stop=True)

        if has_bias:
            for t, b0, b1 in ((a_grz0, "b_gr05_0", "b_gz05_0"),
                              (a_grz1, "b_gr05_1", "b_gz05_1")):
                p = t.shape[0]
                nc.vector.tensor_scalar(t[0:p, 0:F], t[0:p, 0:F],
                                        bv(b0, p), None, op0=ALU.add)
                nc.vector.tensor_scalar(t[0:p, F:F2], t[0:p, F:F2],
                                        bv(b1, p), None, op0=ALU.add)
            nc.vector.tensor_scalar(a_gi0[:], a_gi0[:],
                                    bv("b_gin_0"), None, op0=ALU.add)
            nc.vector.tensor_scalar(a_gi1[:], a_gi1[:],
                                    bv("b_gin_1", 72), None, op0=ALU.add)
            nc.vector.tensor_scalar(a_gh[:, 0:F], a_gh[:, 0:F],
                                    bv("b_ghn_0"), None, op0=ALU.add)
            nc.vector.tensor_scalar(a_gh[0:72, F:F2], a_gh[0:72, F:F2],
                                    bv("b_ghn_1", 72), None, op0=ALU.add)
        t_grz0 = gp.tile([128, F2], BF16, name="t_grz0", tag="t_grz0")
        nc.scalar.activation(t_grz0[:], a_grz0[:], AF.Tanh, scale=0.5)
        t_grz1 = gp.tile([72, F2], BF16, name="t_grz1", tag="t_grz1")
        nc.scalar.activation(t_grz1[:], a_grz1[:], AF.Tanh, scale=0.5)
        # sigma tiles: w_gr/w_gz merged across chunks ([:,0:F] c0, [:,F:F2] c1)
        w_gr = gp.tile([128, F2], BF16, name="w_gr", tag="w_gr")
        nc.vector.tensor_scalar(w_gr[:, 0:F], t_grz0[:, 0:F], 1.0, 0.5,
                                op0=ALU.add, op1=ALU.mult)
        nc.vector.tensor_scalar(w_gr[0:72, F:F2], t_grz1[:, 0:F], 1.0, 0.5,
                                op0=ALU.add, op1=ALU.mult)
        w_gz = gp.tile([128, F2], BF16, name="w_gz", tag="w_gz")
        nc.vector.tensor_scalar(w_gz[:, 0:F], t_grz0[:, F:F2], 1.0, 0.5,
                                op0=ALU.add, op1=ALU.mult)
        nc.vector.tensor_scalar(w_gz[0:72, F:F2], t_grz1[:, F:F2], 1.0, 0.5,
                                op0=ALU.add, op1=ALU.mult)
        tp_g = gp.tile([128, F2], BF16, name="tp_g", tag="tp_g")
        nc.vector.tensor_tensor(tp_g[:], w_gr[:], a_gh[:], op=ALU.mult)
        # u = a_i + r*h_n via identity-matmul accumulation onto a_gi
        mm(a_gi0[:], Ident, tp_g[:, 0:F], start=False, stop=True)
        mm(a_gi1[:], Ident[0:72, 0:72], tp_g[0:72, F:F2],
           start=False, stop=True)
        io[f"w_gz{s}"], io[f"a_gi{s}"] = w_gz, (a_gi0, a_gi1)

    def stage_gen_a2(io, s):
        a_gi0, a_gi1 = io.pop(f"a_gi{s}")
        n_g = gp.tile([128, F2], BF16, name="n_g", tag="n_g")
        nc.scalar.activation(n_g[:, 0:F], a_gi0[:], AF.Tanh)
        nc.scalar.activation(n_g[0:72, F:F2], a_gi1[:], AF.Tanh)
        io[f"n_g{s}"] = n_g

    # ------------- generator GRU: second eltwise + clip -------------
    def stage_gen_b(io, s):
        cs = slice(s * F, (s + 1) * F)
        w_gz, n_g = io.pop(f"w_gz{s}"), io.pop(f"n_g{s}")
        d_g = gp.tile([128, F2], BF16, name="d_g", tag="d_g")
        nc.vector.tensor_tensor(d_g[:, 0:F], io["g1t"][:, cs], n_g[:, 0:F],
                                op=ALU.subtract)
        nc.vector.tensor_tensor(d_g[0:72, F:F2], io["g2t"][0:72, cs],
                                n_g[0:72, F:F2], op=ALU.subtract)
        e_g = gp.tile([128, F2], BF16, name="e_g", tag="e_g")
        nc.vector.tensor_tensor(e_g[:], w_gz[:], d_g[:], op=ALU.mult)
        hp_g = gp.tile([128, F2], BF16, name="hp_g", tag="hp_g")
        nc.vector.tensor_tensor(hp_g[:], e_g[:], n_g[:], op=ALU.add)
        nc.vector.tensor_scalar(io["og1"][:, cs], hp_g[:, 0:F], CLIP, -CLIP,
                                op0=ALU.min, op1=ALU.max)
        nc.vector.tensor_scalar(io["og2"][:, cs], hp_g[0:72, F:F2], CLIP,
                                -CLIP, op0=ALU.min, op1=ALU.max)

    # ------------- factors (batch-major: out free = 64 feats) -------------
    def stage_fac(io, s):
        p_f = ppq.tile([128, F // 2], F32, name="p_f", tag="sq")
        for b in range(4):
            cb = slice(s * F + b * 128, s * F + (b + 1) * 128)
            d = slice(b * 64, (b + 1) * 64)
            mm(p_f[:, d], io["og1"][:, cb], Wf1, start=True, stop=False)
            mm(p_f[:, d], io["og2"][:, cb], Wf2, start=False, stop=True)
        nc.scalar.copy(io["fct"][:, s * (F // 2):(s + 1) * (F // 2)],
                       p_f[:])

    def stage_store(io):
        g, cg = io["g"], io["cg"]
        nc.sync.dma_start(outs["oga"][:, g * 2 * F2:(g + 1) * 2 * F2],
                          io["oga"][:])
        nc.sync.dma_start(outs["og2"][:, cg], io["og2"][:])
        nc.sync.dma_start(outs["gx"][:, cg], io["xt"][0:20, :])
        for s in range(2):
            cot = io.pop(f"cot{s}")
            nc.sync.dma_start(
                outs["cox"][:, (2 * g + s) * F2:(2 * g + s + 1) * F2],
                cot[:, 0:F2])
        nc.sync.dma_start(outs["fct"][:, g * F2 // 2:(g + 1) * F2 // 2],
                          io["fct"][:, 0:F])

    # epsb lives batch-major for a potential bm-co path; here feature-major
    # eps rides in g2t rows 92:96 instead, so epsb load is unused padding-free.
    # (kept: epsb is the cheap [128,32] layout; co uses g2t rows.)

    # Software pipeline over NG groups of 2 tiles each: while group k's
    # generator half runs, group k+1's controller half fills the PE queue.
    def gen_half(io):
        for s in range(2):
            stage_gen_a(io, s)
        for s in range(2):
            stage_gen_a2(io, s)
        for s in range(2):
            stage_gen_b(io, s)
            stage_fac(io, s)
        stage_store(io)

    prev = None
    nxt = stage_load(0)
    for g in range(NG):
        io = nxt
        for s in range(2):
            stage_con_a(io, s)
        if g + 1 < NG:
            nxt = stage_load(g + 1)
        for s in range(2):
            stage_con_a2(io, s)
        if prev is not None:
            for s in range(2):
                stage_gen_a(prev, s)
        for s in range(2):
            stage_con_b(io, s)
            stage_co(io, s)
        nc.sync.dma_start(io["g2t"][72:76, :], io["xt"][0:4, :])
        if prev is not None:
            for s in range(2):
                stage_gen_a2(prev, s)
            for s in range(2):
                stage_gen_b(prev, s)
                stage_fac(prev, s)
            stage_store(prev)
        prev = io
    gen_half(prev)


def _weight_arrays(gen_w_ih, gen_w_hh, gen_b_ih, gen_b_hh,
                   con_w_ih, con_w_hh, con_b_ih, con_b_hh, co_w, co_b, fac_w):
    f = np.float32
    cwT = np.asarray(con_w_ih, f).T          # [320, 384]
    chT = np.asarray(con_w_hh, f).T          # [128, 384]
    gwT = np.asarray(gen_w_ih, f).T          # [20, 600]
    ghT = np.asarray(gen_w_hh, f).T          # [200, 600]
    nrm = np.maximum(np.linalg.norm(np.asarray(fac_w, np.float64), axis=1,
                                    keepdims=True), 1e-12)
    facT = np.asarray(fac_w / nrm, f).T      # [200, 64]

    parts = {
        "Wc1": cwT[0:128], "Wc2": cwT[128:256], "Wc4": cwT[256:320],
        "Wc3rz": chT[:, 0:256], "Wc3n": chT[:, 256:384],
        "Wg1": ghT[0:128],
        "Wg2": np.concatenate([ghT[128:200], gwT], axis=0),
        "Wgx": gwT[:, 400:600],
        "Wco": np.asarray(co_w, f).T,
        "Wf1": facT[0:128], "Wf2": facT[128:200],
        "Ident": np.eye(128, dtype=f),
    }
    wpack = np.zeros((128, WPACK_COLS), dtype=NPBF)
    for nm, (p, c, c0) in _WCOLS.items():
        wpack[0:p, c0:c0 + c] = parts[nm].astype(NPBF)

    cbi = np.asarray(con_b_ih, f)
    cbh = np.asarray(con_b_hh, f)
    gbi = np.asarray(gen_b_ih, f)
    gbh = np.asarray(gen_b_hh, f)
    cob = np.asarray(co_b, f)
    bvec = np.zeros((128, NBCOLS), dtype=f)

    def setb(nm, vals):
        v = np.asarray(vals, f).ravel()
        bvec[0:len(v), _BCOLS[nm]] = v

    setb("b_cr05", 0.5 * (cbi[0:128] + cbh[0:128]))
    setb("b_cz05", 0.5 * (cbi[128:256] + cbh[128:256]))
    setb("b_cin", cbi[256:384])
    setb("b_chn", cbh[256:384])
    setb("b_gr05_0", 0.5 * (gbi[0:128] + gbh[0:128]))
    setb("b_gr05_1", 0.5 * (gbi[128:200] + gbh[128:200]))
    setb("b_gz05_0", 0.5 * (gbi[200:328] + gbh[200:328]))
    setb("b_gz05_1", 0.5 * (gbi[328:400] + gbh[328:400]))
    setb("b_gin_0", gbi[400:528])
    setb("b_gin_1", gbi[528:600])
    setb("b_ghn_0", gbh[400:528])
    setb("b_ghn_1", gbh[528:600])
    setb("b_m", cob[0:4])
    setb("b_v05", 0.5 * cob[4:8])

    has_bias = bool(
        np.any(cbi) or np.any(cbh) or np.any(gbi) or np.any(gbh))
    return {"wpack": wpack, "bvec": bvec}, has_bias


_CACHED = {}


def _build_nc(has_bias=False):
    key = ("v2", has_bias)
    if key in _CACHED:
        return _CACHED[key]
    from contextlib import ExitStack

    nc = bacc.Bacc("TRN2", target_bir_lowering=False, debug=False,
                   num_devices=N_CORES)
    ins = {
        "cc": nc.dram_tensor("cc", [128, 4 * ROWS], BF16,
                             kind="ExternalInput").ap(),
        "m1": nc.dram_tensor("m1", [M1_ROWS, ROWS], BF16,
                             kind="ExternalInput").ap(),
        "wpack": nc.dram_tensor("wpack", [128, WPACK_COLS], BF16,
                                kind="ExternalInput").ap(),
        "bvec": nc.dram_tensor("bvec", [128, NBCOLS], F32,
                               kind="ExternalInput").ap(),
    }
    outs = {
        "oga": nc.dram_tensor("oga", [128, 2 * ROWS], BF16,
                              kind="ExternalOutput").ap(),
        "og2": nc.dram_tensor("og2", [72, ROWS], BF16,
                              kind="ExternalOutput").ap(),
        "gx": nc.dram_tensor("gx", [20, ROWS], BF16,
                             kind="ExternalOutput").ap(),
        "cox": nc.dram_tensor("cox", [4, 2 * ROWS], BF16,
                              kind="ExternalOutput").ap(),
        "fct": nc.dram_tensor("fct", [128, ROWS // 2], BF16,
                              kind="ExternalOutput").ap(),
    }
    with tile.TileContext(nc) as tc:
        with ExitStack() as ctx:
            build_decoder(nc, tc, ctx, ins, outs, has_bias)
    nc.compile()
    _CACHED[key] = nc
    return nc


def pack_inputs(x, h0, eps):
    """Host-side packing of one core's activations (bf16, [feat, rows])."""
    xb = x.astype(NPBF)
    hb = h0.astype(NPBF)
    eb = eps.astype(NPBF)
    blocks = np.stack([xb[:, 0:128].T, xb[:, 128:256].T,
                       hb[:, 200:328].T, hb[:, 0:128].T])  # [4,128,rows]
    cc = np.ascontiguousarray(
        blocks.reshape(4, 128, NG, F2).transpose(1, 2, 0, 3).reshape(
            128, 4 * ROWS))
    m1 = np.concatenate([hb[:, 356:420].T, hb[:, 128:200].T,
                         xb[:, 256:272].T, eb.T], axis=0)
    return {"cc": cc, "m1": np.ascontiguousarray(m1)}


def unpack_outputs(res):
    out = np.empty((ROWS, STATE), dtype=np.float32)
    oga = res["oga"].astype(np.float32).reshape(128, NG, 2, F2)
    out[:, 0:128] = oga[:, :, 0, :].reshape(128, ROWS).T
    out[:, 128:200] = res["og2"].astype(np.float32).T
    out[:, 200:328] = oga[:, :, 1, :].reshape(128, ROWS).T
    gx = res["gx"].astype(np.float32)    # [20, rows]
    cox = res["cox"].astype(np.float32).reshape(4, NST, 2, F)
    out[:, 328:332] = cox[:, :, 0, :].reshape(4, ROWS).T   # mean
    out[:, 332:336] = cox[:, :, 1, :].reshape(4, ROWS).T   # std
    out[:, 336:356] = gx[0:20].T         # gen_input = [con_out, ext]
    # fct[p, st*256 + b*64 + f] = factor[st*512 + b*128 + p, f]
    fct = res["fct"].astype(np.float32)
    out[:, 356:420] = fct.reshape(128, NST, 4, 64).transpose(
        1, 2, 0, 3).reshape(ROWS, 64)
    return out


def kernel(x, h0, eps, gen_w_ih, gen_w_hh, gen_b_ih, gen_b_hh,
           con_w_ih, con_w_hh, con_b_ih, con_b_hh, co_w, co_b, fac_w,
           **run_kwargs):
    x = np.asarray(x, dtype=np.float32)
    h0 = np.asarray(h0, dtype=np.float32)
    eps = np.asarray(eps, dtype=np.float32)
    w, has_bias = _weight_arrays(gen_w_ih, gen_w_hh, gen_b_ih, gen_b_hh,
                                 con_w_ih, con_w_hh, con_b_ih, con_b_hh,
                                 co_w, co_b, fac_w)
    nc = _build_nc(has_bias)

    in_maps = []
    for c in range(N_CORES):
        r0, r1 = c * ROWS, (c + 1) * ROWS
        m = dict(w)
        m.update(pack_inputs(x[r0:r1], h0[r0:r1], eps[r0:r1]))
        in_maps.append(m)

    res = run_bass_kernel_spmd(nc, in_maps, core_ids=list(range(N_CORES)),
                               **run_kwargs)
    out = np.empty((B, STATE), dtype=np.float32)
    for c in range(N_CORES):
        out[c * ROWS:(c + 1) * ROWS] = unpack_outputs(res.results[c])
    if run_kwargs:
        return out, res
    return out


# revision 37
# speedup vs baseline: 1.6002x; 1.0574x over previous
"""Trainium2 Bass kernel for nn_DecoderCell (LFADS decoder cell), v2.

Strategy: pure data parallel over 8 NeuronCores (8192 batch rows each).
On-chip layout is transposed ([feature, batch]); batch rides the free dim in
512-wide compute tiles (1024-wide DMA tiles). All activation I/O is bf16
(halves HBM traffic); matmuls are bf16 (full PE rate at any free size).
K-chunks are packed so each GRU needs the minimum number of matmul
instructions (con 12, gen 14 per 512 cols). The co/fac linears run
batch-major (activations stationary) so their free dim is the tiny feature
count. Sigmoid is synthesized from tanh (one ACT table set: Exp+Tanh);
biases ride ACT bias APs / conditional TS-adds (zero for this problem).

Host side only transposes/casts/shards numpy arrays; all compute on device.
"""

import numpy as np

import concourse.bass as bass
import concourse.tile as tile
from concourse import bacc, mybir
from concourse.bass_utils import run_bass_kernel_spmd

# ---- problem constants (hardcoded; kernel.py must be self-contained) ----
B = 65536
N_CORES = 8
ROWS = B // N_CORES          # 8192 rows per core
F = 512                      # batch tile (free dim) per compute step
NST = ROWS // F              # 16 compute tiles per core
F2 = 2 * F                   # DMA/store tile width
NG = ROWS // F2              # 8 DMA groups per core

GEN = 200
CON = 128
CO = 4
LAT = 64
CIE = 128
EXT = 16
CLIP = 5.0
STATE = 420

F32 = mybir.dt.float32
BF16 = mybir.dt.bfloat16
NPBF = mybir.dt.np(BF16)
AF = mybir.ActivationFunctionType
ALU = mybir.AluOpType

# weight pack column layout (bf16): name -> (rows, cols, col_offset)
_WCOLS = {}
_off = 0
for _nm, _p, _c in (
    ("Wc1", 128, 384), ("Wc2", 128, 384), ("Wc4", 64, 384),
    ("Wc3rz", 128, 256), ("Wc3n", 128, 128),
    ("Wg1", 128, 600), ("Wg2", 92, 600), ("Wgx", 20, 200),
    ("Wco", 128, 8), ("Wf1", 128, 64), ("Wf2", 72, 64),
    ("Ident", 128, 128), ("Identh", 128, 128),
):
    _WCOLS[_nm] = (_p, _c, _off)
    _off += _c
WPACK_COLS = _off

# bias vector pack (f32): name -> column
_BCOLS = {nm: i for i, nm in enumerate(
    ("b_cr05", "b_cz05", "b_cin", "b_chn",
     "b_gr05_0", "b_gr05_1", "b_gz05_0", "b_gz05_1",
     "b_gin_0", "b_gin_1", "b_ghn_0", "b_ghn_1",
     "b_m", "b_v05"))}
NBCOLS = len(_BCOLS)

# m1 packed input rows: [fac 64 | hg1 72 | ext 16 | eps 4]
M1_ROWS = 64 + 72 + 16 + 4   # 156

# Matmul operand bases must be 0/32/64 (32/64 with limited spans), engine-op
# bases 0/32/64/96. g2t holds the 92-row gen rz K-chunk; xt holds the 20-row
# x block (con_out engine-written at base 0), DMA-copied into g2t[72:76].
#   g2t: 0:72 hg1 | 72:76 con_out copy | 76:92 ext
G2_ROWS = 92


def build_decoder(nc: bass.Bass, tc: tile.TileContext, ctx, ins, outs,
                  has_bias: bool):
    wp = ctx.enter_context(tc.tile_pool(name="wp", bufs=1))
    lp = ctx.enter_context(tc.tile_pool(name="lp", bufs=3))
    gp = ctx.enter_context(tc.tile_pool(name="gp", bufs=3))
    op = ctx.enter_context(tc.tile_pool(name="op", bufs=3))
    ppb = ctx.enter_context(tc.tile_pool(name="ppb", bufs=2, space="PSUM"))
    pps = ctx.enter_context(tc.tile_pool(name="pps", bufs=3, space="PSUM"))
    ppq = ctx.enter_context(tc.tile_pool(name="ppq", bufs=1, space="PSUM"))

    wsb = wp.tile([128, WPACK_COLS], BF16, name="wsb")
    nc.sync.dma_start(wsb[:], ins["wpack"][:])
    bvt = wp.tile([128, NBCOLS], F32, name="bvt")
    nc.sync.dma_start(bvt[:], ins["bvec"][:])

    def wv(name):
        p, c, c0 = _WCOLS[name]
        return wsb[0:p, c0:c0 + c]

    def bv(name, p=128):
        return bvt[0:p, _BCOLS[name]:_BCOLS[name] + 1]

    Wc1, Wc2, Wc4 = wv("Wc1"), wv("Wc2"), wv("Wc4")
    Wc3rz, Wc3n = wv("Wc3rz"), wv("Wc3n")
    Wg1, Wg2, Wco = wv("Wg1"), wv("Wg2"), wv("Wco")
    Wgx = wv("Wgx")
    Ident = wv("Ident")
    Identh = wv("Identh")
    Wf1, Wf2 = wv("Wf1"), wv("Wf2")

    mm = nc.tensor.matmul

    # ---------------- per-group (2 tiles) load ----------------
    def stage_load(g):
        cg = slice(g * F2, (g + 1) * F2)
        cct = lp.tile([128, 4 * F2], BF16, name="cct", tag="cct")
        if g == 0:
            # split the first load so the first matmuls start sooner
            for b in range(4):
                nc.sync.dma_start(
                    cct[:, b * F2:(b + 1) * F2],
                    ins["cc"][:, g * 4 * F2 + b * F2:g * 4 * F2 + (b + 1) * F2])
        else:
            nc.sync.dma_start(cct[:],
                              ins["cc"][:, g * 4 * F2:(g + 1) * 4 * F2])
        c1t = cct[:, 0:F2]
        c2t = cct[:, F2:2 * F2]
        c3t = cct[:, 2 * F2:3 * F2]
        g1t = cct[:, 3 * F2:4 * F2]
        c4t = lp.tile([64, F2], BF16, name="c4t", tag="c4t")
        nc.sync.dma_start(c4t[:], ins["m1"][0:64, cg])
        g2t = lp.tile([G2_ROWS, F2], BF16, name="g2t", tag="g2t")
        nc.sync.dma_start(g2t[0:72, :], ins["m1"][64:136, cg])
        nc.sync.dma_start(g2t[76:92, :], ins["m1"][136:152, cg])
        xt = lp.tile([20, F2], BF16, name="xt", tag="xt")
        nc.sync.dma_start(xt[4:20, :], ins["m1"][136:152, cg])
        epst = lp.tile([4, F2], BF16, name="epst", tag="epst")
        nc.sync.dma_start(epst[:], ins["m1"][152:156, cg])
        oga = op.tile([128, 2 * F2], BF16, name="oga", tag="oga")
        og1 = oga[:, 0:F2]
        ogc = oga[:, F2:2 * F2]
        og2 = op.tile([72, F2], BF16, name="og2", tag="og2")
        fct = op.tile([128, F], BF16, name="fct", tag="fct")
        return dict(g=g, cg=cg, c1t=c1t, c2t=c2t, c3t=c3t, g1t=g1t, c4t=c4t,
                    g2t=g2t, xt=xt, epst=epst, oga=oga, ogc=ogc, og1=og1,
                    og2=og2, fct=fct)

    # ------------- controller GRU: matmuls + first eltwise -------------
    def stage_con_a(io, s):
        cs = slice(s * F, (s + 1) * F)
        c1 = io["c1t"][:, cs]
        c2 = io["c2t"][:, cs]
        c3 = io["c3t"][:, cs]
        c4 = io["c4t"][:, cs]
        a_crz = ppb.tile([128, F2], F32, name="a_crz", tag="rz")
        for d, c0 in ((slice(0, F), 0), (slice(F, F2), 128)):
            mm(a_crz[:, d], Wc1[:, c0:c0 + 128], c1, start=True, stop=False)
            mm(a_crz[:, d], Wc2[:, c0:c0 + 128], c2, start=False, stop=False)
            mm(a_crz[:, d], Wc4[:, c0:c0 + 128], c4, start=False, stop=False)
            mm(a_crz[:, d], Wc3rz[:, c0:c0 + 128], c3, start=False, stop=True)
        a_ci = pps.tile([128, F], F32, name="a_ci", tag="sm")
        mm(a_ci[:], Wc1[:, 256:384], c1, start=True, stop=False)
        mm(a_ci[:], Wc2[:, 256:384], c2, start=False, stop=False)
        mm(a_ci[:], Wc4[:, 256:384], c4, start=False, stop=False)
        a_ch = pps.tile([128, F], F32, name="a_ch", tag="sm")
        mm(a_ch[:], Wc3n, c3, start=True, stop=True)

        if has_bias:
            nc.vector.tensor_scalar(a_crz[:, 0:F], a_crz[:, 0:F],
                                    bv("b_cr05"), None, op0=ALU.add)
            nc.vector.tensor_scalar(a_crz[:, F:F2], a_crz[:, F:F2],
                                    bv("b_cz05"), None, op0=ALU.add)
            nc.vector.tensor_scalar(a_ci[:], a_ci[:], bv("b_cin"), None,
                                    op0=ALU.add)
            nc.vector.tensor_scalar(a_ch[:], a_ch[:], bv("b_chn"), None,
                                    op0=ALU.add)
        t_crz = gp.tile([128, F2], BF16, name="t_crz", tag="t_crz")
        nc.scalar.activation(t_crz[:], a_crz[:], AF.Tanh, scale=0.5)
        # sigma_z = (tanh_z + 1)/2 via TS (4x DVE mode on bf16)
        w_cz = gp.tile([128, F], BF16, name="w_cz", tag="w_cz")
        nc.gpsimd.tensor_scalar(w_cz[:], t_crz[:, F:F2], 1.0, 0.5,
                                op0=ALU.add, op1=ALU.mult)
        tp_c = gp.tile([128, F], BF16, name="tp_c", tag="tp_c")
        nc.vector.scalar_tensor_tensor(tp_c[:], t_crz[:, 0:F], 1.0, a_ch[:],
                                       op0=ALU.add, op1=ALU.mult)
        # u = a_i + r*h_n via half-identity matmul accumulation (tp = 2*r*h_n)
        mm(a_ci[:], Identh, tp_c[:], start=False, stop=True)
        io[f"w_cz{s}"], io[f"a_ci{s}"] = w_cz, a_ci

    def stage_con_a2(io, s):
        a_ci = io.pop(f"a_ci{s}")
        n_c = gp.tile([128, F], BF16, name="n_c", tag="n_c")
        nc.scalar.activation(n_c[:], a_ci[:], AF.Tanh)
        io[f"n_c{s}"] = n_c

    # ------------- controller GRU: second eltwise + clip -------------
    def stage_con_b(io, s):
        cs = slice(s * F, (s + 1) * F)
        w_cz, n_c = io.pop(f"w_cz{s}"), io.pop(f"n_c{s}")
        d_c = gp.tile([128, F], BF16, name="d_c", tag="d_c")
        nc.vector.tensor_tensor(d_c[:], io["c3t"][:, cs], n_c[:],
                                op=ALU.subtract)
        e_c = gp.tile([128, F], BF16, name="e_c", tag="e_c")
        nc.vector.tensor_tensor(e_c[:], w_cz[:], d_c[:], op=ALU.mult)
        hp_c = gp.tile([128, F], BF16, name="hp_c", tag="hp_c")
        nc.vector.tensor_tensor(hp_c[:], e_c[:], n_c[:], op=ALU.add)
        nc.vector.tensor_scalar(io["ogc"][:, cs], hp_c[:], CLIP, -CLIP,
                                op0=ALU.min, op1=ALU.max)

    # ------------- controller output sample (feature-major) -------------
    def stage_co(io, s):
        cs = slice(s * F, (s + 1) * F)
        g2t = io["g2t"]
        p_co = ppq.tile([36, F], F32, name="p_co", tag="sq")
        p_cm = p_co[0:4, :]
        p_cv = p_co[32:36, :]
        mm(p_cm, Wco[:, 0:4], io["ogc"][:, cs], start=True, stop=True)
        mm(p_cv, Wco[:, 4:8], io["ogc"][:, cs], start=True, stop=True)
        # cot cols: [mean | std | q], all at partition base 0
        cot = gp.tile([4, 3 * F], BF16, name="cot", tag="cot")
        # std = exp(0.5*logvar + 0.5*b_v); bias AP is free
        nc.scalar.activation(cot[:, F:F2], p_cv, AF.Exp,
                             scale=0.5, bias=bv("b_v05", 4))
        # mean = p_cm + b_m
        nc.vector.tensor_scalar(cot[:, 0:F], p_cm, bv("b_m", 4),
                                None, op0=ALU.add)
        nc.vector.tensor_tensor(cot[:, F2:3 * F], cot[:, F:F2],
                                io["epst"][:, cs], op=ALU.mult)
        # con_out = mean + std*eps (mean already biased, bf16-rounded)
        nc.vector.tensor_tensor(io["xt"][0:4, cs], cot[:, F2:3 * F],
                                cot[:, 0:F], op=ALU.add)
        io[f"cot{s}"] = cot

    # ------------- generator GRU: matmuls + first eltwise -------------
    def stage_gen_a(io, s):
        cs = slice(s * F, (s + 1) * F)
        g1 = io["g1t"][:, cs]
        g2k = io["g2t"][0:92, cs]
        g2h = io["g2t"][0:72, cs]
        g2x = io["xt"][0:20, cs]
        a_grz0 = ppb.tile([128, F2], F32, name="a_grz0", tag="rz")
        a_grz1 = ppb.tile([72, F2], F32, name="a_grz1", tag="rz")
        for d, c0 in ((slice(0, F), 0), (slice(F, F2), 200)):
            mm(a_grz0[:, d], Wg1[:, c0:c0 + 128], g1, start=True, stop=False)
            mm(a_grz0[:, d], Wg2[:, c0:c0 + 128], g2k, start=False, stop=True)
            mm(a_grz1[:, d], Wg1[:, c0 + 128:c0 + 200], g1,
               start=True, stop=False)
            mm(a_grz1[:, d], Wg2[:, c0 + 128:c0 + 200], g2k,
               start=False, stop=True)
        a_gi0 = pps.tile([128, F], F32, name="a_gi0", tag="sm")
        mm(a_gi0[:], Wgx[:, 0:128], g2x, start=True, stop=False)
        a_gi1 = pps.tile([72, F], F32, name="a_gi1", tag="sm")
        mm(a_gi1[:], Wgx[:, 128:200], g2x, start=True, stop=False)
        a_gh = ppb.tile([128, F2], F32, name="a_gh", tag="rz")
        mm(a_gh[:, 0:F], Wg1[:, 400:528], g1, start=True, stop=False)
        mm(a_gh[:, 0:F], Wg2[0:72, 400:528], g2h, start=False, stop=True)
        mm(a_gh[0:72, F:F2], Wg1[:, 528:600], g1, start=True, stop=False)
        mm(a_gh[0:72, F:F2], Wg2[0:72, 528:600], g2h, start=False, stop=True)

        if has_bias:
            for t, b0, b1 in ((a_grz0, "b_gr05_0", "b_gz05_0"),
                              (a_grz1, "b_gr05_1", "b_gz05_1")):
                p = t.shape[0]
                nc.vector.tensor_scalar(t[0:p, 0:F], t[0:p, 0:F],
                                        bv(b0, p), None, op0=ALU.add)
                nc.vector.tensor_scalar(t[0:p, F:F2], t[0:p, F:F2],
                                        bv(b1, p), None, op0=ALU.add)
            nc.vector.tensor_scalar(a_gi0[:], a_gi0[:],
                                    bv("b_gin_0"), None, op0=ALU.add)
            nc.vector.tensor_scalar(a_gi1[:], a_gi1[:],
                                    bv("b_gin_1", 72), None, op0=ALU.add)
            nc.vector.tensor_scalar(a_gh[:, 0:F], a_gh[:, 0:F],
                                    bv("b_ghn_0"), None, op0=ALU.add)
            nc.vector.tensor_scalar(a_gh[0:72, F:F2], a_gh[0:72, F:F2],
                                    bv("b_ghn_1", 72), None, op0=ALU.add)
        t_grz0 = gp.tile([128, F2], BF16, name="t_grz0", tag="t_grz0")
        nc.scalar.activation(t_grz0[:], a_grz0[:], AF.Tanh, scale=0.5)
        t_grz1 = gp.tile([72, F2], BF16, name="t_grz1", tag="t_grz1")
        nc.scalar.activation(t_grz1[:], a_grz1[:], AF.Tanh, scale=0.5)
        # sigma_z merged across chunks ([:,0:F] c0, [:,F:F2] c1); off-chain
        w_gz = gp.tile([128, F2], BF16, name="w_gz", tag="w_gz")
        nc.gpsimd.tensor_scalar(w_gz[:, 0:F], t_grz0[:, F:F2], 1.0, 0.5,
                                op0=ALU.add, op1=ALU.mult)
        nc.gpsimd.tensor_scalar(w_gz[0:72, F:F2], t_grz1[:, F:F2], 1.0, 0.5,
                                op0=ALU.add, op1=ALU.mult)
        tp_g = gp.tile([128, F2], BF16, name="tp_g", tag="tp_g")
        nc.vector.scalar_tensor_tensor(tp_g[:, 0:F], t_grz0[:, 0:F], 1.0,
                                       a_gh[:, 0:F], op0=ALU.add,
                                       op1=ALU.mult)
        nc.vector.scalar_tensor_tensor(tp_g[0:72, F:F2], t_grz1[:, 0:F], 1.0,
                                       a_gh[0:72, F:F2], op0=ALU.add,
                                       op1=ALU.mult)
        # u = a_i + r*h_n via half-identity accumulation (tp = 2*r*h_n)
        mm(a_gi0[:], Identh, tp_g[:, 0:F], start=False, stop=True)
        mm(a_gi1[:], Identh[0:72, 0:72], tp_g[0:72, F:F2],
           start=False, stop=True)
        io[f"w_gz{s}"], io[f"a_gi{s}"] = w_gz, (a_gi0, a_gi1)

    def stage_gen_a2(io, s):
        a_gi0, a_gi1 = io.pop(f"a_gi{s}")
        n_g = gp.tile([128, F2], BF16, name="n_g", tag="n_g")
        nc.scalar.activation(n_g[:, 0:F], a_gi0[:], AF.Tanh)
        nc.scalar.activation(n_g[0:72, F:F2], a_gi1[:], AF.Tanh)
        io[f"n_g{s}"] = n_g

    # ------------- generator GRU: second eltwise + clip -------------
    def stage_gen_b(io, s):
        cs = slice(s * F, (s + 1) * F)
        w_gz, n_g = io.pop(f"w_gz{s}"), io.pop(f"n_g{s}")
        d_g = gp.tile([128, F2], BF16, name="d_g", tag="d_g")
        nc.vector.tensor_tensor(d_g[:, 0:F], io["g1t"][:, cs], n_g[:, 0:F],
                                op=ALU.subtract)
        nc.vector.tensor_tensor(d_g[0:72, F:F2], io["g2t"][0:72, cs],
                                n_g[0:72, F:F2], op=ALU.subtract)
        e_g = gp.tile([128, F2], BF16, name="e_g", tag="e_g")
        nc.vector.tensor_tensor(e_g[:], w_gz[:], d_g[:], op=ALU.mult)
        hp_g = gp.tile([128, F2], BF16, name="hp_g", tag="hp_g")
        nc.vector.tensor_tensor(hp_g[:], e_g[:], n_g[:], op=ALU.add)
        nc.vector.tensor_scalar(io["og1"][:, cs], hp_g[:, 0:F], CLIP, -CLIP,
                                op0=ALU.min, op1=ALU.max)
        nc.vector.tensor_scalar(io["og2"][:, cs], hp_g[0:72, F:F2], CLIP,
                                -CLIP, op0=ALU.min, op1=ALU.max)

    # ------------- factors (batch-major: out free = 64 feats) -------------
    def stage_fac(io, s):
        p_f = ppq.tile([128, F // 2], F32, name="p_f", tag="sq")
        for b in range(4):
            cb = slice(s * F + b * 128, s * F + (b + 1) * 128)
            d = slice(b * 64, (b + 1) * 64)
            mm(p_f[:, d], io["og1"][:, cb], Wf1, start=True, stop=False)
            mm(p_f[:, d], io["og2"][:, cb], Wf2, start=False, stop=True)
        nc.scalar.copy(io["fct"][:, s * (F // 2):(s + 1) * (F // 2)],
                       p_f[:])

    def stage_store(io):
        g, cg = io["g"], io["cg"]
        nc.sync.dma_start(outs["oga"][:, g * 2 * F2:(g + 1) * 2 * F2],
                          io["oga"][:])
        nc.sync.dma_start(outs["og2"][:, cg], io["og2"][:])
        nc.sync.dma_start(outs["gx"][:, cg], io["xt"][0:20, :])
        for s in range(2):
            cot = io.pop(f"cot{s}")
            nc.sync.dma_start(
                outs["cox"][:, (2 * g + s) * F2:(2 * g + s + 1) * F2],
                cot[:, 0:F2])
        nc.sync.dma_start(outs["fct"][:, g * F2 // 2:(g + 1) * F2 // 2],
                          io["fct"][:, 0:F])

    # epsb lives batch-major for a potential bm-co path; here feature-major
    # eps rides in g2t rows 92:96 instead, so epsb load is unused padding-free.
    # (kept: epsb is the cheap [128,32] layout; co uses g2t rows.)

    # Software pipeline over NG groups of 2 tiles each: while group k's
    # generator half runs, group k+1's controller half fills the PE queue.
    def gen_half(io):
        for s in range(2):
            stage_gen_a(io, s)
        for s in range(2):
            stage_gen_a2(io, s)
        for s in range(2):
            stage_gen_b(io, s)
            stage_fac(io, s)
        stage_store(io)

    prev = None
    nxt = stage_load(0)
    for g in range(NG):
        io = nxt
        for s in range(2):
            stage_con_a(io, s)
        if g + 1 < NG:
            nxt = stage_load(g + 1)
        for s in range(2):
            stage_con_a2(io, s)
        if prev is not None:
            for s in range(2):
                stage_gen_a(prev, s)
        for s in range(2):
            stage_con_b(io, s)
            stage_co(io, s)
        nc.sync.dma_start(io["g2t"][72:76, :], io["xt"][0:4, :])
        if prev is not None:
            for s in range(2):
                stage_gen_a2(prev, s)
            for s in range(2):
                stage_gen_b(prev, s)
                stage_fac(prev, s)
            stage_store(prev)
        prev = io
    gen_half(prev)


def _weight_arrays(gen_w_ih, gen_w_hh, gen_b_ih, gen_b_hh,
                   con_w_ih, con_w_hh, con_b_ih, con_b_hh, co_w, co_b, fac_w):
    f = np.float32
    cwT = np.asarray(con_w_ih, f).T          # [320, 384]
    chT = np.asarray(con_w_hh, f).T          # [128, 384]
    gwT = np.asarray(gen_w_ih, f).T          # [20, 600]
    ghT = np.asarray(gen_w_hh, f).T          # [200, 600]
    nrm = np.maximum(np.linalg.norm(np.asarray(fac_w, np.float64), axis=1,
                                    keepdims=True), 1e-12)
    facT = np.asarray(fac_w / nrm, f).T      # [200, 64]

    parts = {
        "Wc1": cwT[0:128], "Wc2": cwT[128:256], "Wc4": cwT[256:320],
        "Wc3rz": chT[:, 0:256], "Wc3n": chT[:, 256:384],
        "Wg1": ghT[0:128],
        "Wg2": np.concatenate([ghT[128:200], gwT], axis=0),
        "Wgx": gwT[:, 400:600],
        "Wco": np.asarray(co_w, f).T,
        "Wf1": facT[0:128], "Wf2": facT[128:200],
        "Ident": np.eye(128, dtype=f),
        "Identh": 0.5 * np.eye(128, dtype=f),
    }
    wpack = np.zeros((128, WPACK_COLS), dtype=NPBF)
    for nm, (p, c, c0) in _WCOLS.items():
        wpack[0:p, c0:c0 + c] = parts[nm].astype(NPBF)

    cbi = np.asarray(con_b_ih, f)
    cbh = np.asarray(con_b_hh, f)
    gbi = np.asarray(gen_b_ih, f)
    gbh = np.asarray(gen_b_hh, f)
    cob = np.asarray(co_b, f)
    bvec = np.zeros((128, NBCOLS), dtype=f)

    def setb(nm, vals):
        v = np.asarray(vals, f).ravel()
        bvec[0:len(v), _BCOLS[nm]] = v

    setb("b_cr05", 0.5 * (cbi[0:128] + cbh[0:128]))
    setb("b_cz05", 0.5 * (cbi[128:256] + cbh[128:256]))
    setb("b_cin", cbi[256:384])
    setb("b_chn", cbh[256:384])
    setb("b_gr05_0", 0.5 * (gbi[0:128] + gbh[0:128]))
    setb("b_gr05_1", 0.5 * (gbi[128:200] + gbh[128:200]))
    setb("b_gz05_0", 0.5 * (gbi[200:328] + gbh[200:328]))
    setb("b_gz05_1", 0.5 * (gbi[328:400] + gbh[328:400]))
    setb("b_gin_0", gbi[400:528])
    setb("b_gin_1", gbi[528:600])
    setb("b_ghn_0", gbh[400:528])
    setb("b_ghn_1", gbh[528:600])
    setb("b_m", cob[0:4])
    setb("b_v05", 0.5 * cob[4:8])

    has_bias = bool(
        np.any(cbi) or np.any(cbh) or np.any(gbi) or np.any(gbh))
    return {"wpack": wpack, "bvec": bvec}, has_bias


_CACHED = {}


def _build_nc(has_bias=False):
    key = ("v2", has_bias)
    if key in _CACHED:
        return _CACHED[key]
    from contextlib import ExitStack

    nc = bacc.Bacc("TRN2", target_bir_lowering=False, debug=False,
                   num_devices=N_CORES)
    ins = {
        "cc": nc.dram_tensor("cc", [128, 4 * ROWS], BF16,
                             kind="ExternalInput").ap(),
        "m1": nc.dram_tensor("m1", [M1_ROWS, ROWS], BF16,
                             kind="ExternalInput").ap(),
        "wpack": nc.dram_tensor("wpack", [128, WPACK_COLS], BF16,
                                kind="ExternalInput").ap(),
        "bvec": nc.dram_tensor("bvec", [128, NBCOLS], F32,
                               kind="ExternalInput").ap(),
    }
    outs = {
        "oga": nc.dram_tensor("oga", [128, 2 * ROWS], BF16,
                              kind="ExternalOutput").ap(),
        "og2": nc.dram_tensor("og2", [72, ROWS], BF16,
                              kind="ExternalOutput").ap(),
        "gx": nc.dram_tensor("gx", [20, ROWS], BF16,
                             kind="ExternalOutput").ap(),
        "cox": nc.dram_tensor("cox", [4, 2 * ROWS], BF16,
                              kind="ExternalOutput").ap(),
        "fct": nc.dram_tensor("fct", [128, ROWS // 2], BF16,
                              kind="ExternalOutput").ap(),
    }
    with tile.TileContext(nc) as tc:
        with ExitStack() as ctx:
            build_decoder(nc, tc, ctx, ins, outs, has_bias)
    nc.compile()
    _CACHED[key] = nc
    return nc


def pack_inputs(x, h0, eps):
    """Host-side packing of one core's activations (bf16, [feat, rows])."""
    xb = x.astype(NPBF)
    hb = h0.astype(NPBF)
    eb = eps.astype(NPBF)
    blocks = np.stack([xb[:, 0:128].T, xb[:, 128:256].T,
                       hb[:, 200:328].T, hb[:, 0:128].T])  # [4,128,rows]
    cc = np.ascontiguousarray(
        blocks.reshape(4, 128, NG, F2).transpose(1, 2, 0, 3).reshape(
            128, 4 * ROWS))
    m1 = np.concatenate([hb[:, 356:420].T, hb[:, 128:200].T,
                         xb[:, 256:272].T, eb.T], axis=0)
    return {"cc": cc, "m1": np.ascontiguousarray(m1)}


def unpack_outputs(res):
    out = np.empty((ROWS, STATE), dtype=np.float32)
    oga = res["oga"].astype(np.float32).reshape(128, NG, 2, F2)
    out[:, 0:128] = oga[:, :, 0, :].reshape(128, ROWS).T
    out[:, 128:200] = res["og2"].astype(np.float32).T
    out[:, 200:328] = oga[:, :, 1, :].reshape(128, ROWS).T
    gx = res["gx"].astype(np.float32)    # [20, rows]
    cox = res["cox"].astype(np.float32).reshape(4, NST, 2, F)
    out[:, 328:332] = cox[:, :, 0, :].reshape(4, ROWS).T   # mean
    out[:, 332:336] = cox[:, :, 1, :].reshape(4, ROWS).T   # std
    out[:, 336:356] = gx[0:20].T         # gen_input = [con_out, ext]
    # fct[p, st*256 + b*64 + f] = factor[st*512 + b*128 + p, f]
    fct = res["fct"].astype(np.float32)
    out[:, 356:420] = fct.reshape(128, NST, 4, 64).transpose(
        1, 2, 0, 3).reshape(ROWS, 64)
    return out


def kernel(x, h0, eps, gen_w_ih, gen_w_hh, gen_b_ih, gen_b_hh,
           con_w_ih, con_w_hh, con_b_ih, con_b_hh, co_w, co_b, fac_w,
           **run_kwargs):
    x = np.asarray(x, dtype=np.float32)
    h0 = np.asarray(h0, dtype=np.float32)
    eps = np.asarray(eps, dtype=np.float32)
    w, has_bias = _weight_arrays(gen_w_ih, gen_w_hh, gen_b_ih, gen_b_hh,
                                 con_w_ih, con_w_hh, con_b_ih, con_b_hh,
                                 co_w, co_b, fac_w)
    nc = _build_nc(has_bias)

    in_maps = []
    for c in range(N_CORES):
        r0, r1 = c * ROWS, (c + 1) * ROWS
        m = dict(w)
        m.update(pack_inputs(x[r0:r1], h0[r0:r1], eps[r0:r1]))
        in_maps.append(m)

    res = run_bass_kernel_spmd(nc, in_maps, core_ids=list(range(N_CORES)),
                               **run_kwargs)
    out = np.empty((B, STATE), dtype=np.float32)
    for c in range(N_CORES):
        out[c * ROWS:(c + 1) * ROWS] = unpack_outputs(res.results[c])
    if run_kwargs:
        return out, res
    return out


# revision 45
# speedup vs baseline: 1.6115x; 1.0071x over previous
"""Trainium2 Bass kernel for nn_DecoderCell (LFADS decoder cell), v2.

Strategy: pure data parallel over 8 NeuronCores (8192 batch rows each).
On-chip layout is transposed ([feature, batch]); batch rides the free dim in
512-wide compute tiles (1024-wide DMA tiles). All activation I/O is bf16
(halves HBM traffic); matmuls are bf16 (full PE rate at any free size).
K-chunks are packed so each GRU needs the minimum number of matmul
instructions (con 12, gen 14 per 512 cols). The co/fac linears run
batch-major (activations stationary) so their free dim is the tiny feature
count. Sigmoid is synthesized from tanh (one ACT table set: Exp+Tanh);
biases ride ACT bias APs / conditional TS-adds (zero for this problem).

Host side only transposes/casts/shards numpy arrays; all compute on device.
"""

import numpy as np

import concourse.bass as bass
import concourse.tile as tile
from concourse import bacc, mybir
from concourse.bass_utils import run_bass_kernel_spmd

# ---- problem constants (hardcoded; kernel.py must be self-contained) ----
B = 65536
N_CORES = 8
ROWS = B // N_CORES          # 8192 rows per core
F = 512                      # batch tile (free dim) per compute step
NST = ROWS // F              # 16 compute tiles per core
F2 = 2 * F                   # DMA/store tile width
NG = ROWS // F2              # 8 DMA groups per core

GEN = 200
CON = 128
CO = 4
LAT = 64
CIE = 128
EXT = 16
CLIP = 5.0
STATE = 420

F32 = mybir.dt.float32
BF16 = mybir.dt.bfloat16
NPBF = mybir.dt.np(BF16)
AF = mybir.ActivationFunctionType
ALU = mybir.AluOpType

# weight pack column layout (bf16): name -> (rows, cols, col_offset)
_WCOLS = {}
_off = 0
for _nm, _p, _c in (
    ("Wc1", 128, 384), ("Wc2", 128, 384), ("Wc4", 64, 384),
    ("Wc3rz", 128, 256), ("Wc3n", 128, 128),
    ("Wg1", 128, 600), ("Wg2", 92, 600), ("Wgx", 20, 200),
    ("Wco", 128, 8), ("Wf1", 128, 64), ("Wf2", 72, 64),
    ("Ident", 128, 128), ("Identh", 128, 128),
):
    _WCOLS[_nm] = (_p, _c, _off)
    _off += _c
WPACK_COLS = _off

# bias vector pack (f32): name -> column
_BCOLS = {nm: i for i, nm in enumerate(
    ("b_cr05", "b_cz05", "b_cin", "b_chn",
     "b_gr05_0", "b_gr05_1", "b_gz05_0", "b_gz05_1",
     "b_gin_0", "b_gin_1", "b_ghn_0", "b_ghn_1",
     "b_m", "b_v05"))}
NBCOLS = len(_BCOLS)

# m1 packed input rows: [fac 64 | hg1 72 | ext 16 | eps 4]
M1_ROWS = 64 + 72 + 16 + 4   # 156

# Matmul operand bases must be 0/32/64 (32/64 with limited spans), engine-op
# bases 0/32/64/96. g2t holds the 92-row gen rz K-chunk; xt holds the 20-row
# x block (con_out engine-written at base 0), DMA-copied into g2t[72:76].
#   g2t: 0:72 hg1 | 72:76 con_out copy | 76:92 ext
G2_ROWS = 92


def build_decoder(nc: bass.Bass, tc: tile.TileContext, ctx, ins, outs,
                  has_bias: bool):
    wp = ctx.enter_context(tc.tile_pool(name="wp", bufs=1))
    lp = ctx.enter_context(tc.tile_pool(name="lp", bufs=3))
    gp = ctx.enter_context(tc.tile_pool(name="gp", bufs=3))
    op = ctx.enter_context(tc.tile_pool(name="op", bufs=3))
    ppb = ctx.enter_context(tc.tile_pool(name="ppb", bufs=2, space="PSUM"))
    pps = ctx.enter_context(tc.tile_pool(name="pps", bufs=3, space="PSUM"))
    ppq = ctx.enter_context(tc.tile_pool(name="ppq", bufs=1, space="PSUM"))

    wsb = wp.tile([128, WPACK_COLS], BF16, name="wsb")
    # staged weight load: first rz-chunk weights, then rest of con, then gen+
    nc.sync.dma_start(wsb[:, 0:384], ins["wpack"][:, 0:384])
    nc.sync.dma_start(wsb[:, 384:1536], ins["wpack"][:, 384:1536])
    nc.sync.dma_start(wsb[:, 1536:], ins["wpack"][:, 1536:])
    bvt = wp.tile([128, NBCOLS], F32, name="bvt")
    nc.sync.dma_start(bvt[:], ins["bvec"][:])

    def wv(name):
        p, c, c0 = _WCOLS[name]
        return wsb[0:p, c0:c0 + c]

    def bv(name, p=128):
        return bvt[0:p, _BCOLS[name]:_BCOLS[name] + 1]

    Wc1, Wc2, Wc4 = wv("Wc1"), wv("Wc2"), wv("Wc4")
    Wc3rz, Wc3n = wv("Wc3rz"), wv("Wc3n")
    Wg1, Wg2, Wco = wv("Wg1"), wv("Wg2"), wv("Wco")
    Wgx = wv("Wgx")
    Ident = wv("Ident")
    Identh = wv("Identh")
    Wf1, Wf2 = wv("Wf1"), wv("Wf2")

    mm = nc.tensor.matmul

    # ---------------- per-group (2 tiles) load ----------------
    def stage_load(g):
        cg = slice(g * F2, (g + 1) * F2)
        cct = lp.tile([128, 4 * F2], BF16, name="cct", tag="cct")
        if g == 0:
            # split the first load so the first matmuls start sooner
            for b in range(4):
                nc.sync.dma_start(
                    cct[:, b * F2:(b + 1) * F2],
                    ins["cc"][:, g * 4 * F2 + b * F2:g * 4 * F2 + (b + 1) * F2])
        else:
            nc.sync.dma_start(cct[:],
                              ins["cc"][:, g * 4 * F2:(g + 1) * 4 * F2])
        c1t = cct[:, 0:F2]
        c2t = cct[:, F2:2 * F2]
        c3t = cct[:, 2 * F2:3 * F2]
        g1t = cct[:, 3 * F2:4 * F2]
        c4t = lp.tile([64, F2], BF16, name="c4t", tag="c4t")
        nc.sync.dma_start(c4t[:], ins["m1"][0:64, cg])
        g2t = lp.tile([G2_ROWS, F2], BF16, name="g2t", tag="g2t")
        nc.sync.dma_start(g2t[0:72, :], ins["m1"][64:136, cg])
        nc.sync.dma_start(g2t[76:92, :], ins["m1"][136:152, cg])
        xt = lp.tile([20, F2], BF16, name="xt", tag="xt")
        nc.sync.dma_start(xt[4:20, :], ins["m1"][136:152, cg])
        epst = lp.tile([4, F2], BF16, name="epst", tag="epst")
        nc.sync.dma_start(epst[:], ins["m1"][152:156, cg])
        oga = op.tile([128, 2 * F2], BF16, name="oga", tag="oga")
        og1 = oga[:, 0:F2]
        ogc = oga[:, F2:2 * F2]
        og2 = op.tile([72, F2], BF16, name="og2", tag="og2")
        fct = op.tile([128, F], BF16, name="fct", tag="fct")
        return dict(g=g, cg=cg, c1t=c1t, c2t=c2t, c3t=c3t, g1t=g1t, c4t=c4t,
                    g2t=g2t, xt=xt, epst=epst, oga=oga, ogc=ogc, og1=og1,
                    og2=og2, fct=fct)

    # ------------- controller GRU: matmuls + first eltwise -------------
    def stage_con_a(io, s):
        cs = slice(s * F, (s + 1) * F)
        c1 = io["c1t"][:, cs]
        c2 = io["c2t"][:, cs]
        c3 = io["c3t"][:, cs]
        c4 = io["c4t"][:, cs]
        a_crz = ppb.tile([128, F2], F32, name="a_crz", tag="rz")
        for d, c0 in ((slice(0, F), 0), (slice(F, F2), 128)):
            mm(a_crz[:, d], Wc1[:, c0:c0 + 128], c1, start=True, stop=False)
            mm(a_crz[:, d], Wc2[:, c0:c0 + 128], c2, start=False, stop=False)
            mm(a_crz[:, d], Wc4[:, c0:c0 + 128], c4, start=False, stop=False)
            mm(a_crz[:, d], Wc3rz[:, c0:c0 + 128], c3, start=False, stop=True)
        a_ci = pps.tile([128, F], F32, name="a_ci", tag="sm")
        mm(a_ci[:], Wc1[:, 256:384], c1, start=True, stop=False)
        mm(a_ci[:], Wc2[:, 256:384], c2, start=False, stop=False)
        mm(a_ci[:], Wc4[:, 256:384], c4, start=False, stop=False)
        a_ch = pps.tile([128, F], F32, name="a_ch", tag="sm")
        mm(a_ch[:], Wc3n, c3, start=True, stop=True)

        if has_bias:
            nc.vector.tensor_scalar(a_crz[:, 0:F], a_crz[:, 0:F],
                                    bv("b_cr05"), None, op0=ALU.add)
            nc.vector.tensor_scalar(a_crz[:, F:F2], a_crz[:, F:F2],
                                    bv("b_cz05"), None, op0=ALU.add)
            nc.vector.tensor_scalar(a_ci[:], a_ci[:], bv("b_cin"), None,
                                    op0=ALU.add)
            nc.vector.tensor_scalar(a_ch[:], a_ch[:], bv("b_chn"), None,
                                    op0=ALU.add)
        t_crz = gp.tile([128, F2], BF16, name="t_crz", tag="t_crz")
        nc.scalar.activation(t_crz[:], a_crz[:], AF.Tanh, scale=0.5)
        # sigma_z = (tanh_z + 1)/2 via TS (4x DVE mode on bf16)
        w_cz = gp.tile([128, F], BF16, name="w_cz", tag="w_cz")
        nc.gpsimd.tensor_scalar(w_cz[:], t_crz[:, F:F2], 1.0, 0.5,
                                op0=ALU.add, op1=ALU.mult)
        tp_c = gp.tile([128, F], BF16, name="tp_c", tag="tp_c")
        nc.vector.scalar_tensor_tensor(tp_c[:], t_crz[:, 0:F], 1.0, a_ch[:],
                                       op0=ALU.add, op1=ALU.mult)
        # u = a_i + r*h_n via half-identity matmul accumulation (tp = 2*r*h_n)
        mm(a_ci[:], Identh, tp_c[:], start=False, stop=True)
        io[f"w_cz{s}"], io[f"a_ci{s}"] = w_cz, a_ci

    def stage_con_a2(io, s):
        a_ci = io.pop(f"a_ci{s}")
        n_c = gp.tile([128, F], BF16, name="n_c", tag="n_c")
        nc.scalar.activation(n_c[:], a_ci[:], AF.Tanh)
        io[f"n_c{s}"] = n_c

    # ------------- controller GRU: second eltwise + clip -------------
    def stage_con_b(io, s):
        cs = slice(s * F, (s + 1) * F)
        w_cz, n_c = io.pop(f"w_cz{s}"), io.pop(f"n_c{s}")
        d_c = gp.tile([128, F], BF16, name="d_c", tag="d_c")
        nc.vector.tensor_tensor(d_c[:], io["c3t"][:, cs], n_c[:],
                                op=ALU.subtract)
        e_c = gp.tile([128, F], BF16, name="e_c", tag="e_c")
        nc.vector.tensor_tensor(e_c[:], w_cz[:], d_c[:], op=ALU.mult)
        hp_c = gp.tile([128, F], BF16, name="hp_c", tag="hp_c")
        nc.vector.tensor_tensor(hp_c[:], e_c[:], n_c[:], op=ALU.add)
        nc.vector.tensor_scalar(io["ogc"][:, cs], hp_c[:], CLIP, -CLIP,
                                op0=ALU.min, op1=ALU.max)

    # ------------- controller output sample (feature-major) -------------
    def stage_co(io, s):
        cs = slice(s * F, (s + 1) * F)
        g2t = io["g2t"]
        p_co = ppq.tile([36, F], F32, name="p_co", tag="sq")
        p_cm = p_co[0:4, :]
        p_cv = p_co[32:36, :]
        mm(p_cm, Wco[:, 0:4], io["ogc"][:, cs], start=True, stop=True)
        mm(p_cv, Wco[:, 4:8], io["ogc"][:, cs], start=True, stop=True)
        # cot cols: [mean | std | q], all at partition base 0
        cot = gp.tile([4, 3 * F], BF16, name="cot", tag="cot")
        # std = exp(0.5*logvar + 0.5*b_v); bias AP is free
        nc.scalar.activation(cot[:, F:F2], p_cv, AF.Exp,
                             scale=0.5, bias=bv("b_v05", 4))
        # mean = p_cm + b_m
        nc.vector.tensor_scalar(cot[:, 0:F], p_cm, bv("b_m", 4),
                                None, op0=ALU.add)
        nc.vector.tensor_tensor(cot[:, F2:3 * F], cot[:, F:F2],
                                io["epst"][:, cs], op=ALU.mult)
        # con_out = mean + std*eps (mean already biased, bf16-rounded)
        nc.vector.tensor_tensor(io["xt"][0:4, cs], cot[:, F2:3 * F],
                                cot[:, 0:F], op=ALU.add)
        io[f"cot{s}"] = cot

    # ------------- generator GRU: matmuls + first eltwise -------------
    def stage_gen_a(io, s):
        cs = slice(s * F, (s + 1) * F)
        g1 = io["g1t"][:, cs]
        g2k = io["g2t"][0:92, cs]
        g2h = io["g2t"][0:72, cs]
        g2x = io["xt"][0:20, cs]
        a_grz0 = ppb.tile([128, F2], F32, name="a_grz0", tag="rz")
        a_grz1 = ppb.tile([72, F2], F32, name="a_grz1", tag="rz")
        for d, c0 in ((slice(0, F), 0), (slice(F, F2), 200)):
            mm(a_grz0[:, d], Wg1[:, c0:c0 + 128], g1, start=True, stop=False)
            mm(a_grz0[:, d], Wg2[:, c0:c0 + 128], g2k, start=False, stop=True)
            mm(a_grz1[:, d], Wg1[:, c0 + 128:c0 + 200], g1,
               start=True, stop=False)
            mm(a_grz1[:, d], Wg2[:, c0 + 128:c0 + 200], g2k,
               start=False, stop=True)
        a_gi0 = pps.tile([128, F], F32, name="a_gi0", tag="sm")
        mm(a_gi0[:], Wgx[:, 0:128], g2x, start=True, stop=False)
        a_gi1 = pps.tile([72, F], F32, name="a_gi1", tag="sm")
        mm(a_gi1[:], Wgx[:, 128:200], g2x, start=True, stop=False)
        a_gh = ppb.tile([128, F2], F32, name="a_gh", tag="rz")
        mm(a_gh[:, 0:F], Wg1[:, 400:528], g1, start=True, stop=False)
        mm(a_gh[:, 0:F], Wg2[0:72, 400:528], g2h, start=False, stop=True)
        mm(a_gh[0:72, F:F2], Wg1[:, 528:600], g1, start=True, stop=False)
        mm(a_gh[0:72, F:F2], Wg2[0:72, 528:600], g2h, start=False, stop=True)

        if has_bias:
            for t, b0, b1 in ((a_grz0, "b_gr05_0", "b_gz05_0"),
                              (a_grz1, "b_gr05_1", "b_gz05_1")):
                p = t.shape[0]
                nc.vector.tensor_scalar(t[0:p, 0:F], t[0:p, 0:F],
                                        bv(b0, p), None, op0=ALU.add)
                nc.vector.tensor_scalar(t[0:p, F:F2], t[0:p, F:F2],
                                        bv(b1, p), None, op0=ALU.add)
            nc.vector.tensor_scalar(a_gi0[:], a_gi0[:],
                                    bv("b_gin_0"), None, op0=ALU.add)
            nc.vector.tensor_scalar(a_gi1[:], a_gi1[:],
                                    bv("b_gin_1", 72), None, op0=ALU.add)
            nc.vector.tensor_scalar(a_gh[:, 0:F], a_gh[:, 0:F],
                                    bv("b_ghn_0"), None, op0=ALU.add)
            nc.vector.tensor_scalar(a_gh[0:72, F:F2], a_gh[0:72, F:F2],
                                    bv("b_ghn_1", 72), None, op0=ALU.add)
        t_grz0 = gp.tile([128, F2], BF16, name="t_grz0", tag="t_grz0")
        nc.scalar.activation(t_grz0[:], a_grz0[:], AF.Tanh, scale=0.5)
        t_grz1 = gp.tile([72, F2], BF16, name="t_grz1", tag="t_grz1")
        nc.scalar.activation(t_grz1[:], a_grz1[:], AF.Tanh, scale=0.5)
        # sigma_z merged across chunks ([:,0:F] c0, [:,F:F2] c1); off-chain
        w_gz = gp.tile([128, F2], BF16, name="w_gz", tag="w_gz")
        nc.gpsimd.tensor_scalar(w_gz[:, 0:F], t_grz0[:, F:F2], 1.0, 0.5,
                                op0=ALU.add, op1=ALU.mult)
        nc.gpsimd.tensor_scalar(w_gz[0:72, F:F2], t_grz1[:, F:F2], 1.0, 0.5,
                                op0=ALU.add, op1=ALU.mult)
        tp_g = gp.tile([128, F2], BF16, name="tp_g", tag="tp_g")
        nc.vector.scalar_tensor_tensor(tp_g[:, 0:F], t_grz0[:, 0:F], 1.0,
                                       a_gh[:, 0:F], op0=ALU.add,
                                       op1=ALU.mult)
        nc.vector.scalar_tensor_tensor(tp_g[0:72, F:F2], t_grz1[:, 0:F], 1.0,
                                       a_gh[0:72, F:F2], op0=ALU.add,
                                       op1=ALU.mult)
        # u = a_i + r*h_n via half-identity accumulation (tp = 2*r*h_n)
        mm(a_gi0[:], Identh, tp_g[:, 0:F], start=False, stop=True)
        mm(a_gi1[:], Identh[0:72, 0:72], tp_g[0:72, F:F2],
           start=False, stop=True)
        io[f"w_gz{s}"], io[f"a_gi{s}"] = w_gz, (a_gi0, a_gi1)

    def stage_gen_a2(io, s):
        a_gi0, a_gi1 = io.pop(f"a_gi{s}")
        n_g = gp.tile([128, F2], BF16, name="n_g", tag="n_g")
        nc.scalar.activation(n_g[:, 0:F], a_gi0[:], AF.Tanh)
        nc.scalar.activation(n_g[0:72, F:F2], a_gi1[:], AF.Tanh)
        io[f"n_g{s}"] = n_g

    # ------------- generator GRU: second eltwise + clip -------------
    def stage_gen_b(io, s):
        cs = slice(s * F, (s + 1) * F)
        w_gz, n_g = io.pop(f"w_gz{s}"), io.pop(f"n_g{s}")
        d_g = gp.tile([128, F2], BF16, name="d_g", tag="d_g")
        nc.vector.tensor_tensor(d_g[:, 0:F], io["g1t"][:, cs], n_g[:, 0:F],
                                op=ALU.subtract)
        nc.vector.tensor_tensor(d_g[0:72, F:F2], io["g2t"][0:72, cs],
                                n_g[0:72, F:F2], op=ALU.subtract)
        e_g = gp.tile([128, F2], BF16, name="e_g", tag="e_g")
        nc.vector.tensor_tensor(e_g[:], w_gz[:], d_g[:], op=ALU.mult)
        hp_g = gp.tile([128, F2], BF16, name="hp_g", tag="hp_g")
        nc.vector.tensor_tensor(hp_g[:], e_g[:], n_g[:], op=ALU.add)
        nc.vector.tensor_scalar(io["og1"][:, cs], hp_g[:, 0:F], CLIP, -CLIP,
                                op0=ALU.min, op1=ALU.max)
        nc.vector.tensor_scalar(io["og2"][:, cs], hp_g[0:72, F:F2], CLIP,
                                -CLIP, op0=ALU.min, op1=ALU.max)

    # ------------- factors (batch-major: out free = 64 feats) -------------
    def stage_fac(io, s):
        p_f = ppq.tile([128, F // 2], F32, name="p_f", tag="sq")
        for b in range(4):
            cb = slice(s * F + b * 128, s * F + (b + 1) * 128)
            d = slice(b * 64, (b + 1) * 64)
            mm(p_f[:, d], io["og1"][:, cb], Wf1, start=True, stop=False)
            mm(p_f[:, d], io["og2"][:, cb], Wf2, start=False, stop=True)
        nc.scalar.copy(io["fct"][:, s * (F // 2):(s + 1) * (F // 2)],
                       p_f[:])

    def stage_store(io):
        g, cg = io["g"], io["cg"]
        nc.sync.dma_start(outs["oga"][:, g * 2 * F2:(g + 1) * 2 * F2],
                          io["oga"][:])
        nc.sync.dma_start(outs["og2"][:, cg], io["og2"][:])
        nc.sync.dma_start(outs["gx"][:, cg], io["xt"][0:20, :])
        for s in range(2):
            cot = io.pop(f"cot{s}")
            nc.sync.dma_start(
                outs["cox"][:, (2 * g + s) * F2:(2 * g + s + 1) * F2],
                cot[:, 0:F2])
        nc.sync.dma_start(outs["fct"][:, g * F2 // 2:(g + 1) * F2 // 2],
                          io["fct"][:, 0:F])

    # epsb lives batch-major for a potential bm-co path; here feature-major
    # eps rides in g2t rows 92:96 instead, so epsb load is unused padding-free.
    # (kept: epsb is the cheap [128,32] layout; co uses g2t rows.)

    # Software pipeline over NG groups of 2 tiles each: while group k's
    # generator half runs, group k+1's controller half fills the PE queue.
    def gen_half(io):
        for s in range(2):
            stage_gen_a(io, s)
        for s in range(2):
            stage_gen_a2(io, s)
        for s in range(2):
            stage_gen_b(io, s)
            stage_fac(io, s)
        stage_store(io)

    prev = None
    nxt = stage_load(0)
    for g in range(NG):
        io = nxt
        for s in range(2):
            stage_con_a(io, s)
        if g + 1 < NG:
            nxt = stage_load(g + 1)
        for s in range(2):
            stage_con_a2(io, s)
        if prev is not None:
            for s in range(2):
                stage_gen_a(prev, s)
        for s in range(2):
            stage_con_b(io, s)
            stage_co(io, s)
        nc.sync.dma_start(io["g2t"][72:76, :], io["xt"][0:4, :])
        if prev is not None:
            for s in range(2):
                stage_gen_a2(prev, s)
            for s in range(2):
                stage_gen_b(prev, s)
                stage_fac(prev, s)
            stage_store(prev)
        prev = io
    gen_half(prev)


def _weight_arrays(gen_w_ih, gen_w_hh, gen_b_ih, gen_b_hh,
                   con_w_ih, con_w_hh, con_b_ih, con_b_hh, co_w, co_b, fac_w):
    f = np.float32
    cwT = np.asarray(con_w_ih, f).T          # [320, 384]
    chT = np.asarray(con_w_hh, f).T          # [128, 384]
    gwT = np.asarray(gen_w_ih, f).T          # [20, 600]
    ghT = np.asarray(gen_w_hh, f).T          # [200, 600]
    nrm = np.maximum(np.linalg.norm(np.asarray(fac_w, np.float64), axis=1,
                                    keepdims=True), 1e-12)
    facT = np.asarray(fac_w / nrm, f).T      # [200, 64]

    parts = {
        "Wc1": cwT[0:128], "Wc2": cwT[128:256], "Wc4": cwT[256:320],
        "Wc3rz": chT[:, 0:256], "Wc3n": chT[:, 256:384],
        "Wg1": ghT[0:128],
        "Wg2": np.concatenate([ghT[128:200], gwT], axis=0),
        "Wgx": gwT[:, 400:600],
        "Wco": np.asarray(co_w, f).T,
        "Wf1": facT[0:128], "Wf2": facT[128:200],
        "Ident": np.eye(128, dtype=f),
        "Identh": 0.5 * np.eye(128, dtype=f),
    }
    wpack = np.zeros((128, WPACK_COLS), dtype=NPBF)
    for nm, (p, c, c0) in _WCOLS.items():
        wpack[0:p, c0:c0 + c] = parts[nm].astype(NPBF)

    cbi = np.asarray(con_b_ih, f)
    cbh = np.asarray(con_b_hh, f)
    gbi = np.asarray(gen_b_ih, f)
    gbh = np.asarray(gen_b_hh, f)
    cob = np.asarray(co_b, f)
    bvec = np.zeros((128, NBCOLS), dtype=f)

    def setb(nm, vals):
        v = np.asarray(vals, f).ravel()
        bvec[0:len(v), _BCOLS[nm]] = v

    setb("b_cr05", 0.5 * (cbi[0:128] + cbh[0:128]))
    setb("b_cz05", 0.5 * (cbi[128:256] + cbh[128:256]))
    setb("b_cin", cbi[256:384])
    setb("b_chn", cbh[256:384])
    setb("b_gr05_0", 0.5 * (gbi[0:128] + gbh[0:128]))
    setb("b_gr05_1", 0.5 * (gbi[128:200] + gbh[128:200]))
    setb("b_gz05_0", 0.5 * (gbi[200:328] + gbh[200:328]))
    setb("b_gz05_1", 0.5 * (gbi[328:400] + gbh[328:400]))
    setb("b_gin_0", gbi[400:528])
    setb("b_gin_1", gbi[528:600])
    setb("b_ghn_0", gbh[400:528])
    setb("b_ghn_1", gbh[528:600])
    setb("b_m", cob[0:4])
    setb("b_v05", 0.5 * cob[4:8])

    has_bias = bool(
        np.any(cbi) or np.any(cbh) or np.any(gbi) or np.any(gbh))
    return {"wpack": wpack, "bvec": bvec}, has_bias


_CACHED = {}


def _build_nc(has_bias=False):
    key = ("v2", has_bias)
    if key in _CACHED:
        return _CACHED[key]
    from contextlib import ExitStack

    nc = bacc.Bacc("TRN2", target_bir_lowering=False, debug=False,
                   num_devices=N_CORES)
    ins = {
        "cc": nc.dram_tensor("cc", [128, 4 * ROWS], BF16,
                             kind="ExternalInput").ap(),
        "m1": nc.dram_tensor("m1", [M1_ROWS, ROWS], BF16,
                             kind="ExternalInput").ap(),
        "wpack": nc.dram_tensor("wpack", [128, WPACK_COLS], BF16,
                                kind="ExternalInput").ap(),
        "bvec": nc.dram_tensor("bvec", [128, NBCOLS], F32,
                               kind="ExternalInput").ap(),
    }
    outs = {
        "oga": nc.dram_tensor("oga", [128, 2 * ROWS], BF16,
                              kind="ExternalOutput").ap(),
        "og2": nc.dram_tensor("og2", [72, ROWS], BF16,
                              kind="ExternalOutput").ap(),
        "gx": nc.dram_tensor("gx", [20, ROWS], BF16,
                             kind="ExternalOutput").ap(),
        "cox": nc.dram_tensor("cox", [4, 2 * ROWS], BF16,
                              kind="ExternalOutput").ap(),
        "fct": nc.dram_tensor("fct", [128, ROWS // 2], BF16,
                              kind="ExternalOutput").ap(),
    }
    with tile.TileContext(nc) as tc:
        with ExitStack() as ctx:
            build_decoder(nc, tc, ctx, ins, outs, has_bias)
    nc.compile()
    _CACHED[key] = nc
    return nc


def pack_inputs(x, h0, eps):
    """Host-side packing of one core's activations (bf16, [feat, rows])."""
    xb = x.astype(NPBF)
    hb = h0.astype(NPBF)
    eb = eps.astype(NPBF)
    blocks = np.stack([xb[:, 0:128].T, xb[:, 128:256].T,
                       hb[:, 200:328].T, hb[:, 0:128].T])  # [4,128,rows]
    cc = np.ascontiguousarray(
        blocks.reshape(4, 128, NG, F2).transpose(1, 2, 0, 3).reshape(
            128, 4 * ROWS))
    m1 = np.concatenate([hb[:, 356:420].T, hb[:, 128:200].T,
                         xb[:, 256:272].T, eb.T], axis=0)
    return {"cc": cc, "m1": np.ascontiguousarray(m1)}


def unpack_outputs(res):
    out = np.empty((ROWS, STATE), dtype=np.float32)
    oga = res["oga"].astype(np.float32).reshape(128, NG, 2, F2)
    out[:, 0:128] = oga[:, :, 0, :].reshape(128, ROWS).T
    out[:, 128:200] = res["og2"].astype(np.float32).T
    out[:, 200:328] = oga[:, :, 1, :].reshape(128, ROWS).T
    gx = res["gx"].astype(np.float32)    # [20, rows]
    cox = res["cox"].astype(np.float32).reshape(4, NST, 2, F)
    out[:, 328:332] = cox[:, :, 0, :].reshape(4, ROWS).T   # mean
    out[:, 332:336] = cox[:, :, 1, :].reshape(4, ROWS).T   # std
    out[:, 336:356] = gx[0:20].T         # gen_input = [con_out, ext]
    # fct[p, st*256 + b*64 + f] = factor[st*512 + b*128 + p, f]
    fct = res["fct"].astype(np.float32)
    out[:, 356:420] = fct.reshape(128, NST, 4, 64).transpose(
        1, 2, 0, 3).reshape(ROWS, 64)
    return out


def kernel(x, h0, eps, gen_w_ih, gen_w_hh, gen_b_ih, gen_b_hh,
           con_w_ih, con_w_hh, con_b_ih, con_b_hh, co_w, co_b, fac_w,
           **run_kwargs):
    x = np.asarray(x, dtype=np.float32)
    h0 = np.asarray(h0, dtype=np.float32)
    eps = np.asarray(eps, dtype=np.float32)
    w, has_bias = _weight_arrays(gen_w_ih, gen_w_hh, gen_b_ih, gen_b_hh,
                                 con_w_ih, con_w_hh, con_b_ih, con_b_hh,
                                 co_w, co_b, fac_w)
    nc = _build_nc(has_bias)

    in_maps = []
    for c in range(N_CORES):
        r0, r1 = c * ROWS, (c + 1) * ROWS
        m = dict(w)
        m.update(pack_inputs(x[r0:r1], h0[r0:r1], eps[r0:r1]))
        in_maps.append(m)

    res = run_bass_kernel_spmd(nc, in_maps, core_ids=list(range(N_CORES)),
                               **run_kwargs)
    out = np.empty((B, STATE), dtype=np.float32)
    for c in range(N_CORES):
        out[c * ROWS:(c + 1) * ROWS] = unpack_outputs(res.results[c])
    if run_kwargs:
        return out, res
    return out
